# revision 44
# baseline (speedup 1.0000x reference)
"""Two-layer GCN (PyG GCNConv x2 + ReLU) on 8 Trainium2 NeuronCores.

Sharding: nodes are balance-relabeled and partitioned across the 8 cores
(12544 each incl. pad; N padded 100000 -> 100352 = 8*98*128). Each core:
  S0: h1 = x_shard @ W1 (bf16 matmul), p1 = dinv*h1 -> bf16 table shard;
      written quarter-by-quarter, each quarter AllGathered as soon as ready
      (4 chunked AllGathers -> chunk-major replicated table, overlapping
      the collective with S0 tail + L1 head).
  L1: for each dst-block of 128 nodes, gather p1[src] rows for its incoming
      non-self edges (dma_gather, int16 indices per <=25600-row segment
      aligned with the AG chunks), build 0/1 selector tiles on DVE with an
      all-packed-AP is_equal (2x DVE mode), segment-sum via TensorE matmuls
      in PSUM; the self-loop term is added with one identity matmul from the
      local table block (no gather). Epilogue relu(dinv*agg + b1)*dinv ->
      bf16 table2 shard, again AllGathered in 4 quarter chunks that fire
      mid-L1 and overlap with the L1 tail / L2 head.
  L2: same gather/selector pass over table2 (feature-major accumulation),
      then out = (dinv*agg2) @ W2 + b2 -> fp32 output shard.
Host reassembles the 8 output shards and inverts the relabeling.

Edges are grouped per (core, dst-block, src-chunk) with chunk counts made
uniform across cores so a single SPMD program serves all 8 cores; padding
slots gather row 0 and carry a -1 dst that the selector maps to zero.
"""

import hashlib
import sys

for _p in ("/opt/trn_rl_repo",):
    if _p not in sys.path:
        sys.path.insert(0, _p)

import numpy as np
import ml_dtypes

import concourse.bass as bass  # noqa: F401  (engine types via nc)
import concourse.bacc as bacc
import concourse.mybir as mybir
import concourse.tile as tile

BF16 = mybir.dt.bfloat16
F32 = mybir.dt.float32
I16 = mybir.dt.int16
I32 = mybir.dt.int32

P = 128
NCORES = 8
F1 = 128
F2 = 64
NQ = 4


class CFG:
    def __init__(self, N, IN_DIM, SBSZ=8, MSPAN=640, PHASES=2,
                 NOGATHER=False, NOONEHOT=False, SKIPAG=False,
                 GMAX=8, QROT=4, PSB=2, NOMM=False, LOCALTAB=True,
                 ZB1=True, ZB2=True, WG=3):
        # WG: sb-groups per wave. Blocks of a wave keep persistent PSUM
        # accumulators across the 4 src-quarters so gather tiles free per
        # quarter (not per 4-quarter group), and AllGathers interleave with
        # wave-0 quarters.
        self.WG = WG
        # ZB1/ZB2: bias vectors known to be all-zero (host-checked) -> the
        # whole epilogue collapses to one ACT op per block (scale folded in).
        self.ZB1 = ZB1
        self.ZB2 = ZB2
        self.PHASES = PHASES
        self.NOGATHER = NOGATHER
        self.NOONEHOT = NOONEHOT
        self.SKIPAG = SKIPAG
        self.GMAX = GMAX
        self.QROT = QROT
        self.PSB = PSB
        self.NOMM = NOMM
        self.LOCALTAB = LOCALTAB
        self.N = N
        self.NPAD = NCORES * 98 * P            # 100352
        self.SH = self.NPAD // NCORES          # 12544
        self.NB = self.SH // P                 # 98
        # per-shard quarter row counts (block-aligned), sum = SH. The last
        # quarter is oversized so its edge cells target ~604 of a 640
        # (5-chunk) cap while the others target ~479 of 512 (4 chunks) --
        # this gives the balance packer ~1.3 sigma of slack per cell.
        # Small quarters FIRST: AG chunk 0 of each table covers fewer rows,
        # completes sooner, and unblocks the L1/L2 gather streams earlier.
        self.QROWS = [2944, 2944, 2944, 3712]
        self.QLO = np.concatenate([[0], np.cumsum(self.QROWS)]).astype(np.int64)
        # per-chunk full-table segment sizes (8 * qrows), all < 32768
        self.SEGSZ = [NCORES * r for r in self.QROWS]
        self.IN_DIM = IN_DIM
        self.SBSZ = SBSZ
        self.MSPAN = MSPAN


DEFAULT_CFG = CFG(N=100000, IN_DIM=512)

_cache = {}


def _balanced_perm(deg, cfg):
    """Relabel nodes so per-(core,block) in-degree sums are balanced.

    Returns perm: old node id -> new node id in [0, NPAD).
    New id layout: core c owns [c*SH, (c+1)*SH); block b of core c is
    rows [c*SH + b*P, c*SH + (b+1)*P).
    """
    NPAD, SH, NB = cfg.NPAD, cfg.SH, cfg.NB
    nbins = NCORES * NB
    order = np.argsort(-deg, kind="stable")  # heavy nodes first
    # snake-deal node ranks into bins: round r covers bins in alternating order
    nodes_per_bin = P
    perm = np.empty(cfg.N, dtype=np.int64)
    fwd = np.arange(nbins)
    bwd = fwd[::-1]
    pos_in_bin = np.zeros(nbins, dtype=np.int64)
    idx = 0
    r = 0
    npts = len(order)
    while idx < npts:
        bins = fwd if (r % 2 == 0) else bwd
        take = min(nbins, npts - idx)
        sel = order[idx:idx + take]
        b = bins[:take]
        # new id: bin b -> core = b // NB, block = b % NB
        core = b // NB
        blk = b % NB
        perm[sel] = core * SH + blk * P + pos_in_bin[b]
        pos_in_bin[b] += 1
        idx += take
        r += 1
    assert pos_in_bin.max() <= nodes_per_bin
    return perm


def _refine_perm(perm, src0, dst0, cfg):
    """Re-bin nodes within each (core, quarter) to equalize the per
    (core, dst-block, src-quarter) edge-cell counts, minimizing the padded
    chunk count sum(ceil(max_core(cell)/128)).

    Quarter membership (and hence every edge's src-quarter) is invariant
    under these moves, so cell profiles can be computed once.
    """
    SH, NB, NPAD = cfg.SH, cfg.NB, cfg.NPAD
    qlo = cfg.QLO
    qblk = [int(q) // P for q in qlo]          # block index at quarter starts
    src = perm[src0]
    dst = perm[dst0]
    # per-node in-profile over src quarters (invariant)
    sq = np.searchsorted(qlo[1:-1], src % SH, side="right")
    pin = np.zeros((NPAD, NQ), dtype=np.int64)
    np.add.at(pin, (dst, sq), 1)

    new_perm_pos = np.arange(NPAD, dtype=np.int64)  # new position per new id

    def pack(order_ids, prof, nblk, caps):
        """Greedy: place nodes (given order) into nblk blocks, cap P nodes
        each, minimizing hinge over caps [nblk, NQ]."""
        fill = np.zeros((nblk, NQ), dtype=np.int64)
        cnt = np.zeros(nblk, dtype=np.int64)
        assign = np.empty(len(order_ids), dtype=np.int64)
        for i, v in enumerate(order_ids):
            p = prof[i]
            over = np.maximum(fill + p - caps, 0) - np.maximum(fill - caps, 0)
            score = over.sum(axis=1).astype(np.float64)
            # tie-break: prefer emptier blocks (balance node counts)
            score += cnt * 1e-6
            score[cnt >= P] = np.inf
            b = int(np.argmin(score))
            assign[i] = b
            fill[b] += p
            cnt[b] += 1
        return assign, fill

    # two rounds: first against the (5,4,4,4)-chunk grid matched to the
    # skewed quarter sizes, then against the chunk grid actually paid for
    # (max over cores), letting overflow consolidate into paid cells.
    caps_all = np.tile(np.array([[4, 4, 4, 5]], dtype=np.int64) * P, (NB, 1))
    for rnd in range(2):
        fills = np.zeros((NCORES, NB, NQ), dtype=np.int64)
        for c in range(NCORES):
            for Q in range(NQ):
                blo, bhi = qblk[Q], qblk[Q + 1]
                ids = np.arange(c * SH + qlo[Q], c * SH + qlo[Q + 1])
                prof = pin[ids]
                o = np.argsort(-prof.sum(axis=1), kind="stable")
                ids, prof = ids[o], prof[o]
                assign, fill = pack(ids, prof, bhi - blo, caps_all[blo:bhi])
                fills[c, blo:bhi] = fill
                # positions: stable order within block
                order2 = np.argsort(assign, kind="stable")
                srt = assign[order2]
                startb = np.searchsorted(srt, np.arange(bhi - blo))
                posn = c * SH + (blo + srt) * P + (np.arange(len(ids)) -
                                                   startb[srt])
                new_perm_pos[ids[order2]] = posn
        caps_all = np.ceil(fills.max(axis=0) / P).astype(np.int64) * P
    # compose: old id -> phase1 new id -> refined position
    return new_perm_pos[perm]


def _plan(src, dst, cfg):
    """Group (non-self) edges by (core, dst-block, src-chunk).

    src/dst are NEW (relabeled) node ids. Returns the uniform chunk plan.
    """
    SH, NB = cfg.SH, cfg.NB
    c = dst // SH
    dloc = dst - c * SH
    b = dloc // P
    dl = dloc - b * P
    # src chunk + index within chunk segment
    sc = src % SH
    q = np.searchsorted(cfg.QLO[1:-1], sc, side="right")
    iseg = (src // SH) * np.asarray(cfg.QROWS)[q] + (sc - cfg.QLO[q])
    key = ((c * NB + b) * NQ + q).astype(np.int64)
    counts = np.bincount(key, minlength=NCORES * NB * NQ).reshape(NCORES, NB, NQ)
    order = np.argsort(key, kind="stable")
    starts = np.zeros(NCORES * NB * NQ + 1, dtype=np.int64)
    np.cumsum(counts.reshape(-1), out=starts[1:])
    nch = np.ceil(counts.max(axis=0) / P).astype(np.int64)  # [NB, NQ] uniform
    sbs = [list(range(i, min(i + cfg.SBSZ, NB))) for i in range(0, NB, cfg.SBSZ)]
    return {
        "order": order, "starts": starts, "counts": counts,
        "nch": nch, "sbs": sbs, "iseg": iseg, "dl": dl,
    }


def _core_arrays(plan, core, cfg):
    """Build idx (gather stream, (sb,q,b) order) + dstl ((wave,q,b)-major)."""
    nch, sbs = plan["nch"], plan["sbs"]
    order, starts = plan["order"], plan["starts"]
    iseg, dl = plan["iseg"], plan["dl"]
    NB = cfg.NB

    cell_iv = {}
    cell_dv = {}
    for b in range(NB):
        for q in range(NQ):
            n_ch = nch[b][q]
            if n_ch == 0:
                continue
            k = (core * NB + b) * NQ + q
            sl = order[starts[k]:starts[k + 1]]
            # ascending source rows within the cell: consecutive gather
            # descriptors walk the segment in address order (HBM page
            # locality for the latency-bound random reads)
            sl = sl[np.argsort(iseg[sl], kind="stable")]
            pad = n_ch * P - len(sl)
            cell_iv[(b, q)] = np.concatenate([iseg[sl], np.zeros(pad, np.int64)])
            cell_dv[(b, q)] = np.concatenate([dl[sl], np.full(pad, -1, np.int64)])

    idx_cols = []   # per (sb,q): [16, gn*8] int16 segments
    for sb in sbs:
        for q in range(NQ):
            vals = [cell_iv[(b, q)] for b in sb if (b, q) in cell_iv]
            if vals:
                v = np.concatenate(vals)
                idx_cols.append(v.reshape(-1, 16).T.astype(np.int16))
    idx1 = np.tile(np.concatenate(idx_cols, axis=1), (8, 1)) if idx_cols else \
        np.zeros((128, 0), np.int16)

    # dstl columns in (wave, q, block) order so each wave-quarter's one-hot
    # selector is built from one contiguous slice
    wb = cfg.WG * cfg.SBSZ
    dstl_parts = []
    for w0 in range(0, NB, wb):
        for q in range(NQ):
            for b in range(w0, min(w0 + wb, NB)):
                if (b, q) in cell_dv:
                    dstl_parts.append(cell_dv[(b, q)].reshape(-1, P).T)
    dstl = np.concatenate(dstl_parts, axis=1).astype(np.float32)
    return np.ascontiguousarray(idx1), \
        np.ascontiguousarray(dstl.astype(ml_dtypes.bfloat16))


def _build_program(plan, cfg):
    SH, NB = cfg.SH, cfg.NB
    IN_DIM, SBSZ, MSPAN = cfg.IN_DIM, cfg.SBSZ, cfg.MSPAN
    KC = IN_DIM // P
    nch, sbs = plan["nch"], plan["sbs"]
    nchb = nch.sum(axis=1)                      # chunks per block
    totch = int(nchb.sum())
    nchb_max = int(nchb.max())
    # gather-stream offsets per (sbi, q) and per-block offsets within groups
    goff = {}
    boff = {}
    off = 0
    for sbi, sb in enumerate(sbs):
        for q in range(NQ):
            gn = int(sum(nch[b][q] for b in sb))
            goff[(sbi, q)] = (off, gn)
            o = 0
            for b in sb:
                boff[(b, q)] = o
                o += int(nch[b][q])
            off += gn
    gn_max = max(gn for (_, gn) in goff.values())

    # waves of WG sb-groups; dstl columns are (wave, q, block)-major, so a
    # group's columns for one quarter are one contiguous run
    waves = [list(enumerate(sbs))[i:i + cfg.WG]
             for i in range(0, len(sbs), cfg.WG)]
    sgoff = {}   # (sbi, q) -> dstl col offset of the group's quarter-q run
    off_d = 0
    for wi, wv in enumerate(waves):
        for q in range(NQ):
            for sbi, sb in wv:
                sgoff[(sbi, q)] = off_d
                off_d += int(sum(nch[b][q] for b in sb))

    nc = bacc.Bacc("TRN2", target_bir_lowering=False, debug=False,
                   num_devices=NCORES, num_swdge_queues=min(4, max(1, cfg.QROT)))
    t_xT = nc.declare_dram_parameter("xT", [IN_DIM, SH], BF16, isOutput=False)
    t_W1 = nc.declare_dram_parameter("W1", [IN_DIM, F1], BF16, isOutput=False)
    t_W2 = nc.declare_dram_parameter("W2", [F1, F2], BF16, isOutput=False)
    t_b1b = nc.declare_dram_parameter("b1b", [P, F1], F32, isOutput=False)
    t_b2b = nc.declare_dram_parameter("b2b", [P, F2], F32, isOutput=False)
    t_degc = nc.declare_dram_parameter("degc", [P, NB], F32, isOutput=False)
    t_degr = (None if cfg.ZB2 else
              nc.declare_dram_parameter("degr", [NB * P], F32, isOutput=False))
    t_idx = nc.declare_dram_parameter("idx", [P, totch * 8], I16, isOutput=False)
    t_dstl = nc.declare_dram_parameter("dstl", [P, totch], BF16, isOutput=False)
    t_y = nc.declare_dram_parameter("y", [SH, F2], F32, isOutput=True)

    # Local (non-Shared) collective outputs: dma_gather reads from the
    # Shared scratchpad run ~28% slower per descriptor (~+1ms over the
    # kernel), and the collectives have plenty of slack to take the
    # non-Shared path instead.
    _aspace = "Local" if cfg.LOCALTAB else "Shared"
    tab1_fq = [nc.dram_tensor(f"tab1_full{q}", [cfg.SEGSZ[q], F1], BF16,
                              addr_space=_aspace) for q in range(NQ)]
    tab2_fq = [nc.dram_tensor(f"tab2_full{q}", [cfg.SEGSZ[q], F1], BF16,
                              addr_space=_aspace) for q in range(NQ)]

    with tile.TileContext(nc) as tc:
        with (
            tc.tile_pool(name="dram", bufs=1, space="DRAM") as dram,
            tc.tile_pool(name="consts", bufs=1) as consts,
            tc.tile_pool(name="sb", bufs=3) as pool,
            tc.tile_pool(name="stage", bufs=2) as stage,
            tc.tile_pool(name="psum", bufs=2, space="PSUM") as psum,
        ):
            tab1_shard = dram.tile([SH, F1], BF16)
            tab2_shard = dram.tile([SH, F1], BF16)

            # ---- constants
            iota_i = consts.tile([P, P], I32)
            nc.gpsimd.iota(iota_i[:], pattern=[[1, P]], base=0, channel_multiplier=0)
            iota_p = consts.tile([P, P], I32)
            nc.gpsimd.iota(iota_p[:], pattern=[[0, P]], base=0, channel_multiplier=1)
            iota_bf = consts.tile([P, P], BF16)
            nc.vector.tensor_copy(iota_bf[:], iota_i[:])
            ident = consts.tile([P, P], BF16)
            nc.vector.tensor_tensor(out=ident[:], in0=iota_i[:], in1=iota_p[:],
                                    op=mybir.AluOpType.is_equal)
            iota_rep = consts.tile([P, P, gn_max], BF16)
            nc.vector.tensor_copy(
                iota_rep[:],
                iota_bf[:, :, None].to_broadcast([P, P, gn_max]))

            W1_sb = consts.tile([P, KC, F1], BF16)
            nc.sync.dma_start(out=W1_sb[:],
                              in_=t_W1[:].rearrange("(c p) f -> p c f", p=P))
            W2_bf = consts.tile([P, F2], BF16)
            nc.sync.dma_start(out=W2_bf[:], in_=t_W2[:])
            b1b = consts.tile([P, F1], F32)
            nc.sync.dma_start(out=b1b[:], in_=t_b1b[:])
            b2b = consts.tile([P, F2], F32)
            nc.sync.dma_start(out=b2b[:], in_=t_b2b[:])

            degc = consts.tile([P, NB], F32)
            nc.sync.dma_start(out=degc[:], in_=t_degc[:])
            sq = consts.tile([P, NB], F32)
            nc.scalar.sqrt(sq[:], degc[:])
            dinvc = consts.tile([P, NB], F32)
            nc.vector.reciprocal(dinvc[:], sq[:])
            # dinv^2 per (node, block): relu(dinv*agg)*dinv == relu(dinv2*agg)
            dinv2c = consts.tile([P, NB], F32)
            nc.vector.reciprocal(dinv2c[:], degc[:])

            dinvb = None
            if not cfg.ZB2:
                dinvb = consts.tile([P, NB * P], BF16)
                DSPAN = 1568
                for dspan in range(0, NB * P, DSPAN):
                    dw = min(DSPAN, NB * P - dspan)
                    degb_t = pool.tile([P, DSPAN], F32, tag="degb")
                    nc.sync.dma_start(
                        out=degb_t[:, :dw],
                        in_=t_degr[None, dspan:dspan + dw].to_broadcast([P, dw]))
                    sqb_t = pool.tile([P, DSPAN], F32, tag="sqb")
                    nc.scalar.sqrt(sqb_t[:, :dw], degb_t[:, :dw])
                    rec_t = pool.tile([P, DSPAN], F32, tag="recb")
                    nc.vector.reciprocal(rec_t[:, :dw], sqb_t[:, :dw])
                    nc.vector.tensor_copy(dinvb[:, dspan:dspan + dw], rec_t[:, :dw])

            # ---- S0: h1 = x @ W1 (node-major), p1 = dinv*h1 -> tab1_shard
            # quarter-by-quarter; AllGather each quarter as soon as written.
            for q in range(NQ):
                qlo, qhi = int(cfg.QLO[q]), int(cfg.QLO[q + 1])
                for s0 in range(qlo, qhi, MSPAN):
                    mw = min(MSPAN, qhi - s0)
                    nsub = mw // P
                    xt = pool.tile([P, KC, MSPAN], BF16, tag="xT", bufs=2)
                    nc.sync.dma_start(
                        out=xt[:, :, :mw],
                        in_=t_xT[:, s0:s0 + mw].rearrange("(c p) m -> p c m", p=P))
                    p1s = stage.tile([P, MSPAN // P, F1], BF16, tag="p1s")
                    for sub in range(nsub):
                        moff = sub * P
                        hps = psum.tile([P, F1], F32, tag="aux")
                        for kc in range(KC):
                            nc.tensor.matmul(
                                out=hps[:],
                                lhsT=xt[:, kc, moff:moff + P],
                                rhs=W1_sb[:, kc, :],
                                start=(kc == 0), stop=(kc == KC - 1))
                        B = (s0 + moff) // P
                        nc.scalar.mul(p1s[:, sub, :], hps[:], dinvc[:, B:B + 1])
                    nc.sync.dma_start(
                        out=tab1_shard[s0:s0 + mw, :].rearrange(
                            "(c p) f -> p c f", p=P),
                        in_=p1s[:, :nsub, :])
                # AG1 dispatch deferred into the L1 pass (quarter-major head
                # interleave): see ag1_fire below.

            # ---- aggregation pass (shared for L1/L2)
            qctr = [0]  # global gather-queue rotation (balanced across groups)

            def agg_pass(layer, tabq, tab_shard, out_cb, ag_fire=None,
                         ag2_drain=None):
                mtiles_all = {}

                def emit_gathers(sbi, q):
                    off, gn = goff[(sbi, q)]
                    if gn == 0:
                        return
                    idxt = pool.tile([P, gn_max * 8], I16, tag="idx",
                                     bufs=8)
                    nc.scalar.dma_start(
                        out=idxt[:, :gn * 8],
                        in_=t_idx[:, off * 8:(off + gn) * 8])
                    # deep-buffer the gather stream: with only ~1 group
                    # of tiles the gathers inherit every downstream
                    # bubble (isolated gather rate is ~3x the in-kernel
                    # rate at bufs=5)
                    mt = pool.tile([P, gn_max, F1], BF16, tag="mq",
                                   bufs=8)
                    if cfg.NOGATHER:
                        nc.gpsimd.memset(mt[:, :gn, :], 0.5)
                    else:
                        # HW wedges above 1024 idxs/call (65 ring
                        # entries); cap chunks per call
                        GMAX = cfg.GMAX
                        for g0 in range(0, gn, GMAX):
                            gw = min(GMAX, gn - g0)
                            nc.gpsimd.dma_gather(
                                out_ap=mt[:, g0:g0 + gw, :],
                                in_ap=tabq[q][:],
                                idxs_ap=idxt[:, g0 * 8:(g0 + gw) * 8],
                                num_idxs=gw * P, num_idxs_reg=gw * P,
                                elem_size=F1,
                                queue_num=qctr[0] % cfg.QROT)
                            qctr[0] += 1
                    mtiles_all[(sbi, q)] = mt

                # wave processing: each wave's blocks accumulate into
                # persistent PSUM tiles across the 4 quarters; gather tiles
                # free per (group, quarter). Wave 0 interleaves the AG
                # dispatches between quarters so the Pool stream never queues
                # a gather behind a later quarter's AG input-wait.
                nwave_b = cfg.WG * SBSZ
                for wi, wv in enumerate(waves):
                    # PSUM is 8 banks of 2KB: pack 4 block-accumulators per
                    # bank tile; 6 bank tiles cover a 24-block wave.
                    # start=True zeroes the WHOLE 2KB zero-region, so only
                    # the bank's first matmul starts (siblings accumulate
                    # onto the zeroed region); only its last matmul stops.
                    wblocks = [b for _, sb in wv for b in sb]
                    aggs = {}
                    bankid = {}
                    bank_started = {}
                    bank_last = {}
                    for i4 in range(0, len(wblocks), 4):
                        bank = psum.tile([P, 4, P], F32, tag="agg",
                                         bufs=6, name="aggbk")
                        grp = wblocks[i4:i4 + 4]
                        for j, b in enumerate(grp):
                            aggs[b] = bank[:, j, :]
                            bankid[b] = i4 // 4
                        bank_started[i4 // 4] = False
                        bank_last[grp[-1]] = True
                    for q in range(NQ):
                        if ag_fire is not None and wi == 0:
                            ag_fire(q)
                        if ag2_drain is not None and q == 2:
                            ag2_drain()
                        for sbi, sb in wv:
                            emit_gathers(sbi, q)
                        if cfg.NOMM:
                            continue
                        for sbi, sb in wv:
                            mt = mtiles_all.pop((sbi, q), None)
                            if mt is None:
                                continue
                            _, gn = goff[(sbi, q)]
                            do = sgoff[(sbi, q)]
                            dsb = pool.tile([P, gn_max], BF16, tag="dstl",
                                            bufs=3)
                            nc.scalar.dma_start(
                                out=dsb[:, :gn], in_=t_dstl[:, do:do + gn])
                            oh = pool.tile([P, P, gn_max], BF16, tag="oh",
                                           bufs=3)
                            if cfg.NOONEHOT:
                                nc.vector.memset(oh[:, :, :gn], 0.001)
                            else:
                                # all-packed APs -> DVE 2x mode
                                nc.vector.tensor_tensor(
                                    out=oh[:, :, :gn],
                                    in0=dsb[:, None, :gn].to_broadcast(
                                        [P, P, gn]),
                                    in1=iota_rep[:, :, :gn],
                                    op=mybir.AluOpType.is_equal)
                            for b in sb:
                                for i in range(int(nch[b][q])):
                                    m = mt[:, boff[(b, q)] + i, :]
                                    o = oh[:, :, boff[(b, q)] + i]
                                    agg = aggs[b]
                                    st = not bank_started[bankid[b]]
                                    if layer == 1:
                                        nc.tensor.matmul(
                                            out=agg[:], lhsT=o, rhs=m,
                                            start=st, stop=False)
                                    else:
                                        nc.tensor.matmul(
                                            out=agg[:], lhsT=m, rhs=o,
                                            start=st, stop=False)
                                    bank_started[bankid[b]] = True
                    if cfg.NOMM:
                        continue
                    # self-loop matmuls first (per bank, in order), THEN the
                    # ACT epilogue reads: interleaving mm(b)/read(b) would
                    # serialize mm(b+1) behind read(b) via the shared bank
                    # tile's dependency tracking
                    for sbi, sb in wv:
                        tbsb = pool.tile([P, SBSZ, F1], BF16, tag="tblk",
                                         bufs=2)
                        nc.sync.dma_start(
                            out=tbsb[:, :len(sb), :],
                            in_=tab_shard[sb[0] * P:(sb[-1] + 1) * P,
                                          :].rearrange("(c p) f -> p c f",
                                                       p=P))
                        for b in sb:
                            tblk = tbsb[:, b - sb[0], :]
                            agg = aggs[b]
                            st = not bank_started[bankid[b]]
                            bank_started[bankid[b]] = True
                            sp = bank_last.get(b, False)
                            # self-loop: agg += I^T @ tblk (node-major) or
                            # tblk^T @ I (feature-major)
                            if layer == 1:
                                nc.tensor.matmul(
                                    out=agg[:], lhsT=ident[:], rhs=tblk[:],
                                    start=st, stop=sp)
                            else:
                                nc.tensor.matmul(
                                    out=agg[:], lhsT=tblk[:], rhs=ident[:],
                                    start=st, stop=sp)
                    for b in wblocks:
                        out_cb(b, aggs.pop(b))

            # ---- L1: node-major agg; epilogue -> tab2_shard (+ chunked AG2)
            l1_stage = {}
            qmark = np.searchsorted(cfg.QLO[1:], (np.arange(NB) + 1) * P)
            qend = set(int(q) // P - 1 for q in cfg.QLO[1:])
            # each AG2 chunk becomes pending once its quarter's t2 rows are
            # flushed (epilogues run at wave end, so queue at the wave whose
            # last block covers the quarter end)
            ag2_at = {}
            nwb = cfg.WG * SBSZ
            for qq, qe in enumerate(sorted(qend)):
                we = min(((qe // nwb) + 1) * nwb - 1, NB - 1)
                ag2_at.setdefault(we, []).append(qq)
            ag2_pending = []

            def ag1_fire(q):
                if cfg.SKIPAG:
                    return
                qlo, qhi = int(cfg.QLO[q]), int(cfg.QLO[q + 1])
                nc.gpsimd.collective_compute(
                    "AllGather", mybir.AluOpType.bypass,
                    ins=[tab1_shard[qlo:qhi, :].opt()],
                    outs=[tab1_fq[q][:].opt()],
                    replica_groups=[list(range(NCORES))])
            l1_plo = [0]  # first block not yet flushed to DRAM

            def l1_out(b, agg):
                if b % SBSZ == 0:
                    l1_stage[b // SBSZ] = stage.tile([P, SBSZ, F1], BF16,
                                                     tag="t2", name="t2")
                t2 = l1_stage[b // SBSZ]
                if cfg.ZB1:
                    # t2 = relu(dinv*agg + 0)*dinv = relu(dinv2*agg); one ACT
                    nc.scalar.activation(
                        t2[:, b % SBSZ, :], agg[:],
                        mybir.ActivationFunctionType.Relu,
                        scale=dinv2c[:, b:b + 1])
                else:
                    v = pool.tile([P, F1], F32, tag="v")
                    nc.vector.scalar_tensor_tensor(
                        out=v[:], in0=agg[:], scalar=dinvc[:, b:b + 1],
                        in1=b1b[:], op0=mybir.AluOpType.mult,
                        op1=mybir.AluOpType.add)
                    r = pool.tile([P, F1], F32, tag="r")
                    nc.scalar.activation(r[:], v[:],
                                         mybir.ActivationFunctionType.Relu)
                    nc.vector.tensor_scalar_mul(t2[:, b % SBSZ, :], r[:],
                                                dinvc[:, b:b + 1])
                if b % SBSZ == SBSZ - 1 or b == NB - 1 or b in qend:
                    plo = l1_plo[0]
                    nfb = b - plo + 1
                    nc.sync.dma_start(
                        out=tab2_shard[plo * P:(plo + nfb) * P, :].rearrange(
                            "(c p) f -> p c f", p=P),
                        in_=t2[:, plo % SBSZ:plo % SBSZ + nfb, :])
                    l1_plo[0] = b + 1
                # queue pending AG2 chunks; drained at the next wave's q2
                # point (so the t2 flush has long landed and the Pool stall
                # is nil)
                if cfg.PHASES >= 2 and not cfg.SKIPAG and b in ag2_at:
                    ag2_pending.extend(ag2_at[b])

            def ag2_drain():
                for q in ag2_pending:
                    qlo, qhi = int(cfg.QLO[q]), int(cfg.QLO[q + 1])
                    nc.gpsimd.collective_compute(
                        "AllGather", mybir.AluOpType.bypass,
                        ins=[tab2_shard[qlo:qhi, :].opt()],
                        outs=[tab2_fq[q][:].opt()],
                        replica_groups=[list(range(NCORES))])
                ag2_pending.clear()

            if cfg.PHASES >= 1:
                agg_pass(1, tab1_fq, tab1_shard, l1_out, ag_fire=ag1_fire,
                         ag2_drain=ag2_drain)
                ag2_drain()  # fire the final (q3) AG2 chunk

            # ---- L2: feature-major agg; epilogue -> y
            l2_stage = {}

            def l2_out(b, agg):
                w = pool.tile([P, P], BF16, tag="w")
                if cfg.ZB2:
                    # defer the dinv_d scale to after the W2 matmul (it is a
                    # per-dst scalar); PSUM->SBUF cast rides the ACT copy
                    nc.scalar.copy(w[:], agg[:])
                else:
                    nc.vector.tensor_tensor(
                        out=w[:], in0=agg[:], in1=dinvb[:, b * P:(b + 1) * P],
                        op=mybir.AluOpType.mult)
                o2f = psum.tile([P, F1], F32, tag="aux")
                o2 = o2f[:, :F2]
                nc.tensor.matmul(out=o2, lhsT=w[:], rhs=W2_bf[:],
                                 start=True, stop=True)
                if b % SBSZ == 0:
                    l2_stage[b // SBSZ] = stage.tile([P, SBSZ, F2], F32,
                                                     tag="ys", name="ys")
                ys = l2_stage[b // SBSZ]
                if cfg.ZB2:
                    nc.scalar.mul(ys[:, b % SBSZ, :], o2[:],
                                  dinvc[:, b:b + 1])
                else:
                    nc.vector.tensor_add(ys[:, b % SBSZ, :], o2[:], b2b[:])
                if b % SBSZ == SBSZ - 1 or b == NB - 1:
                    blo = (b // SBSZ) * SBSZ
                    nfb = b - blo + 1
                    nc.scalar.dma_start(
                        out=t_y[blo * P:(blo + nfb) * P, :].rearrange(
                            "(c p) f -> p c f", p=P),
                        in_=ys[:, :nfb, :])

            if cfg.PHASES >= 2:
                agg_pass(2, tab2_fq, tab2_shard, l2_out)
            if cfg.NOMM:
                zt = pool.tile([P, F2], F32, tag="dbg0")
                nc.vector.memset(zt[:], 0.0)
                for bb in range(0, SH, P):
                    nc.scalar.dma_start(out=t_y[bb:bb + P, :], in_=zt[:])
            if cfg.PHASES < 2:
                # debug exit: y <- copy of tab1_full0 head
                dbt = pool.tile([P, F2], BF16, tag="dbgb")
                nc.sync.dma_start(out=dbt[:], in_=tab1_fq[0][0:P, 0:F2])
                dbg = pool.tile([P, F2], F32, tag="dbg")
                nc.vector.tensor_copy(dbg[:], dbt[:])
                for bb in range(0, SH, P):
                    nc.scalar.dma_start(out=t_y[bb:bb + P, :], in_=dbg[:])

    nc.compile()
    return nc


def _prep(x, edge_index, W1, b1, W2, b2, cfg=DEFAULT_CFG):
    N, SH, NB, NPAD = cfg.N, cfg.SH, cfg.NB, cfg.NPAD
    src0 = np.asarray(edge_index[0], dtype=np.int64)
    dst0 = np.asarray(edge_index[1], dtype=np.int64)
    # degree includes the self loop (matches reference)
    deg0 = (np.bincount(dst0, minlength=N) + 1).astype(np.float32)
    perm = _balanced_perm(deg0, cfg)
    perm = _refine_perm(perm, src0, dst0, cfg)
    src = perm[src0]
    dst = perm[dst0]
    degp = np.ones(NPAD, dtype=np.float32)  # pad nodes: deg 1 (no edges)
    degp[perm] = deg0
    plan = _plan(src, dst, cfg)

    xb = np.asarray(x, dtype=np.float32)
    W1b = np.asarray(W1, dtype=ml_dtypes.bfloat16)
    W2b = np.asarray(W2, dtype=ml_dtypes.bfloat16)
    b1b = np.ascontiguousarray(np.tile(np.asarray(b1, np.float32)[None, :], (P, 1)))
    b2b = np.ascontiguousarray(np.tile(np.asarray(b2, np.float32)[None, :], (P, 1)))

    # x rows in new order: xp[newid] = x[oldid]
    invp = np.empty(NPAD, dtype=np.int64)
    invp.fill(0)
    invp[perm] = np.arange(N)
    has = np.zeros(NPAD, dtype=bool)
    has[perm] = True

    in_maps = []
    for c in range(NCORES):
        rows = invp[c * SH:(c + 1) * SH]
        mask = has[c * SH:(c + 1) * SH]
        xs = xb[rows] * mask[:, None]
        xT = np.ascontiguousarray(xs.T.astype(ml_dtypes.bfloat16))
        degsh = degp[c * SH:(c + 1) * SH]
        degc = np.ascontiguousarray(degsh.reshape(NB, P).T)
        idx1, dstl = _core_arrays(plan, c, cfg)
        in_maps.append({
            "xT": xT, "W1": W1b, "W2": W2b, "b1b": b1b, "b2b": b2b,
            "degc": degc, "degr": degsh, "idx": idx1, "dstl": dstl,
        })
    return plan, in_maps, perm


def _get_program(plan, cfg=DEFAULT_CFG):
    key = (hashlib.sha256(plan["nch"].tobytes()).hexdigest() +
           f"{cfg.N}_{cfg.PHASES}_{cfg.NOGATHER}_{cfg.NOONEHOT}_{cfg.SKIPAG}_{cfg.GMAX}_{cfg.QROT}_{cfg.SBSZ}_{cfg.PSB}_{cfg.NOMM}_{cfg.LOCALTAB}_{cfg.ZB1}_{cfg.ZB2}_{cfg.WG}_v4")
    if key not in _cache:
        _cache[key] = _build_program(plan, cfg)
    return _cache[key]


def _make_runner(nc, cfg):
    """Persistent jitted SPMD executor (mirrors bass2jax.run_bass_via_pjrt's
    multi-core path) so repeated calls reuse the compiled NEFF."""
    import jax
    from jax.sharding import Mesh, PartitionSpec
    from jax.experimental.shard_map import shard_map
    from concourse import bass2jax as b2j

    b2j.install_neuronx_cc_hook()
    assert nc.dbg_addr is None
    partition_name = (nc.partition_id_tensor.name
                      if nc.partition_id_tensor else None)

    in_names, out_names, out_avals = [], [], []
    for alloc in nc.m.functions[0].allocations:
        if not isinstance(alloc, mybir.MemoryLocationSet):
            continue
        name = alloc.memorylocations[0].name
        if alloc.kind == "ExternalInput":
            if name != partition_name:
                in_names.append(name)
        elif alloc.kind == "ExternalOutput":
            out_names.append(name)
            out_avals.append(jax.core.ShapedArray(
                tuple(alloc.tensor_shape), mybir.dt.np(alloc.dtype)))
    n_params = len(in_names)
    n_outs = len(out_names)
    all_names = in_names + out_names
    if partition_name is not None:
        all_names = all_names + [partition_name]
    donate = tuple(range(n_params, n_params + n_outs))

    def _body(*args):
        operands = list(args)
        if partition_name is not None:
            operands.append(b2j.partition_id_tensor())
        outs = b2j._bass_exec_p.bind(
            *operands,
            out_avals=tuple(out_avals),
            in_names=tuple(all_names),
            out_names=tuple(out_names),
            lowering_input_output_aliases=(),
            sim_require_finite=True,
            sim_require_nnan=True,
            nc=nc,
        )
        return tuple(outs)

    devices = jax.devices()[:NCORES]
    mesh = Mesh(np.asarray(devices), ("core",))
    sharded = jax.jit(
        shard_map(_body, mesh=mesh,
                  in_specs=(PartitionSpec("core"),) * (n_params + n_outs),
                  out_specs=(PartitionSpec("core"),) * n_outs,
                  check_rep=False),
        donate_argnums=donate, keep_unused=True)
    return {
        "fn": sharded, "in_names": in_names, "out_names": out_names,
        "out_avals": out_avals, "mesh": mesh,
    }


def _runner_args(runner, in_maps):
    concat_in = [
        np.concatenate([np.asarray(in_maps[c][k]) for c in range(NCORES)], 0)
        for k in runner["in_names"]
    ]
    zeros = [
        np.zeros((NCORES * a.shape[0],) + tuple(a.shape[1:]), a.dtype)
        for a in runner["out_avals"]
    ]
    return concat_in, zeros


def _get_runner(plan, cfg=DEFAULT_CFG):
    key = ("runner_" + hashlib.sha256(plan["nch"].tobytes()).hexdigest() +
           f"{cfg.N}_{cfg.PHASES}_{cfg.NOGATHER}_{cfg.NOONEHOT}_{cfg.SKIPAG}_{cfg.GMAX}_{cfg.QROT}_{cfg.SBSZ}_{cfg.PSB}_{cfg.NOMM}_{cfg.LOCALTAB}_{cfg.ZB1}_{cfg.ZB2}_{cfg.WG}_v4")
    if key not in _cache:
        _cache[key] = _make_runner(_get_program(plan, cfg), cfg)
    return _cache[key]


def _cfg_for(b1, b2):
    zb1 = bool(np.all(np.asarray(b1) == 0))
    zb2 = bool(np.all(np.asarray(b2) == 0))
    if zb1 and zb2:
        return DEFAULT_CFG
    return CFG(N=100000, IN_DIM=512, ZB1=zb1, ZB2=zb2)


def kernel(x, edge_index, W1, b1, W2, b2):
    cfg = _cfg_for(b1, b2)
    plan, in_maps, perm = _prep(x, edge_index, W1, b1, W2, b2, cfg)
    runner = _get_runner(plan, cfg)
    concat_in, zeros = _runner_args(runner, in_maps)
    outs = runner["fn"](*concat_in, *zeros)
    y = np.asarray(outs[runner["out_names"].index("y")]).reshape(cfg.NPAD, F2)
    return np.ascontiguousarray(y[perm])


def _floor_overhead(iters=10):
    """Per-call dispatch floor of this PJRT/axon path (trivial program)."""
    import time
    import jax
    from jax.sharding import NamedSharding, PartitionSpec
    import concourse.tile as tile_mod

    if "floor_nc" not in _cache:
        nc = bacc.Bacc("TRN2", target_bir_lowering=False, debug=False,
                       num_devices=NCORES)
        x = nc.declare_dram_parameter("x", [P, P], F32, isOutput=False)
        y = nc.declare_dram_parameter("y", [P, P], F32, isOutput=True)
        with tile_mod.TileContext(nc) as tc:
            with tc.tile_pool(name="sbuf", bufs=2) as pool:
                t = pool.tile([P, P], F32)
                nc.sync.dma_start(out=t[:], in_=x[:])
                t2 = pool.tile([P, P], F32)
                nc.vector.tensor_scalar_mul(t2[:], t[:], 3.0)
                nc.sync.dma_start(out=y[:], in_=t2[:])
        nc.compile()
        _cache["floor_nc"] = _make_runner(nc, None)
    runner = _cache["floor_nc"]
    in_maps = [{"x": np.zeros((P, P), np.float32)} for _ in range(NCORES)]
    concat_in, zeros = _runner_args(runner, in_maps)
    sh = NamedSharding(runner["mesh"], PartitionSpec("core"))
    dev_in = [jax.device_put(a, sh) for a in concat_in]
    zsets = [[jax.device_put(z, sh) for z in zeros] for _ in range(iters + 1)]
    jax.block_until_ready([dev_in, zsets])
    jax.block_until_ready(runner["fn"](*dev_in, *zsets[0]))
    ts = []
    for i in range(iters):
        t0 = time.perf_counter()
        jax.block_until_ready(runner["fn"](*dev_in, *zsets[i + 1]))
        ts.append(time.perf_counter() - t0)
    return min(ts)


def benchmark(inputs, iters=5):
    """Device-time estimate via pipelined-dispatch slope.

    The axon/PJRT channel latency is large and bimodal, so single-call
    wall-minus-floor is unusable. Instead dispatch n back-to-back
    executions (device runs them contiguously) and fit the slope:
    est = (min T(n_hi) - min T(n_lo)) / (n_hi - n_lo), which cancels the
    per-batch channel overhead.
    """
    import time
    import jax
    from jax.sharding import NamedSharding, PartitionSpec

    cfg = _cfg_for(inputs["b1"], inputs["b2"])
    plan, in_maps, _perm = _prep(**inputs, cfg=cfg)
    runner = _get_runner(plan, cfg)
    concat_in, zeros = _runner_args(runner, in_maps)
    sh = NamedSharding(runner["mesh"], PartitionSpec("core"))
    dev_in = [jax.device_put(a, sh) for a in concat_in]

    def run_n(n):
        zsets = [[jax.device_put(z, sh) for z in zeros] for _ in range(n)]
        jax.block_until_ready(zsets)
        t0 = time.perf_counter()
        outs = [runner["fn"](*dev_in, *zsets[i]) for i in range(n)]
        jax.block_until_ready(outs)
        return time.perf_counter() - t0

    run_n(1)  # warmup (compile)
    n_lo, n_hi = 1, 17
    t_lo, t_hi = [], []
    reps = max(4, (iters + 1) // 2)
    for _ in range(reps):
        t_lo.append(run_n(n_lo))
        t_hi.append(run_n(n_hi))
    est = (min(t_hi) - min(t_lo)) / (n_hi - n_lo)
    floor = min(t_lo)
    raw = min(t_lo)
    return {"raw_ns": int(raw * 1e9), "floor_ns": int(floor * 1e9),
            "est_ns": int(max(est, raw * 0.001) * 1e9)}



# revision 53
# speedup vs baseline: 1.0123x; 1.0123x over previous
"""Two-layer GCN (PyG GCNConv x2 + ReLU) on 8 Trainium2 NeuronCores.

Sharding: nodes are balance-relabeled and partitioned across the 8 cores
(12544 each incl. pad; N padded 100000 -> 100352 = 8*98*128). Each core:
  S0: h1 = x_shard @ W1 (bf16 matmul), p1 = dinv*h1 -> bf16 table shard;
      written quarter-by-quarter, each quarter AllGathered as soon as ready
      (4 chunked AllGathers -> chunk-major replicated table, overlapping
      the collective with S0 tail + L1 head).
  L1: for each dst-block of 128 nodes, gather p1[src] rows for its incoming
      non-self edges (dma_gather, int16 indices per <=25600-row segment
      aligned with the AG chunks), build 0/1 selector tiles on DVE with an
      all-packed-AP is_equal (2x DVE mode), segment-sum via TensorE matmuls
      in PSUM; the self-loop term is added with one identity matmul from the
      local table block (no gather). Epilogue relu(dinv*agg + b1)*dinv ->
      bf16 table2 shard, again AllGathered in 4 quarter chunks that fire
      mid-L1 and overlap with the L1 tail / L2 head.
  L2: same gather/selector pass over table2 (feature-major accumulation),
      then out = (dinv*agg2) @ W2 + b2 -> fp32 output shard.
Host reassembles the 8 output shards and inverts the relabeling.

Edges are grouped per (core, dst-block, src-chunk) with chunk counts made
uniform across cores so a single SPMD program serves all 8 cores; padding
slots gather row 0 and carry a -1 dst that the selector maps to zero.
"""

import hashlib
import sys

for _p in ("/opt/trn_rl_repo",):
    if _p not in sys.path:
        sys.path.insert(0, _p)

import numpy as np
import ml_dtypes

import concourse.bass as bass  # noqa: F401  (engine types via nc)
import concourse.bacc as bacc
import concourse.mybir as mybir
import concourse.tile as tile

BF16 = mybir.dt.bfloat16
F32 = mybir.dt.float32
I16 = mybir.dt.int16
I32 = mybir.dt.int32

P = 128
NCORES = 8
F1 = 128
F2 = 64
NQ = 4


class CFG:
    def __init__(self, N, IN_DIM, SBSZ=6, MSPAN=1280, PHASES=2,
                 NOGATHER=False, NOONEHOT=False, SKIPAG=False,
                 GMAX=8, QROT=4, PSB=2, NOMM=False, LOCALTAB=True,
                 ZB1=True, ZB2=True, WG=3):
        # WG: sb-groups per wave. Blocks of a wave keep persistent PSUM
        # accumulators across the 4 src-quarters so gather tiles free per
        # quarter (not per 4-quarter group), and AllGathers interleave with
        # wave-0 quarters.
        self.WG = WG
        # ZB1/ZB2: bias vectors known to be all-zero (host-checked) -> the
        # whole epilogue collapses to one ACT op per block (scale folded in).
        self.ZB1 = ZB1
        self.ZB2 = ZB2
        self.PHASES = PHASES
        self.NOGATHER = NOGATHER
        self.NOONEHOT = NOONEHOT
        self.SKIPAG = SKIPAG
        self.GMAX = GMAX
        self.QROT = QROT
        self.PSB = PSB
        self.NOMM = NOMM
        self.LOCALTAB = LOCALTAB
        self.N = N
        self.NPAD = NCORES * 98 * P            # 100352
        self.SH = self.NPAD // NCORES          # 12544
        self.NB = self.SH // P                 # 98
        # per-shard quarter row counts (block-aligned), sum = SH. The last
        # quarter is oversized so its edge cells target ~604 of a 640
        # (5-chunk) cap while the others target ~479 of 512 (4 chunks) --
        # this gives the balance packer ~1.3 sigma of slack per cell.
        # Small quarters FIRST: AG chunk 0 of each table covers fewer rows,
        # completes sooner, and unblocks the L1/L2 gather streams earlier.
        self.QROWS = [2944, 2944, 2944, 3712]
        self.QLO = np.concatenate([[0], np.cumsum(self.QROWS)]).astype(np.int64)
        # per-chunk full-table segment sizes (8 * qrows), all < 32768
        self.SEGSZ = [NCORES * r for r in self.QROWS]
        self.IN_DIM = IN_DIM
        self.SBSZ = SBSZ
        self.MSPAN = MSPAN


DEFAULT_CFG = CFG(N=100000, IN_DIM=512)

_cache = {}


def _balanced_perm(deg, cfg):
    """Relabel nodes so per-(core,block) in-degree sums are balanced.

    Returns perm: old node id -> new node id in [0, NPAD).
    New id layout: core c owns [c*SH, (c+1)*SH); block b of core c is
    rows [c*SH + b*P, c*SH + (b+1)*P).
    """
    NPAD, SH, NB = cfg.NPAD, cfg.SH, cfg.NB
    nbins = NCORES * NB
    order = np.argsort(-deg, kind="stable")  # heavy nodes first
    # snake-deal node ranks into bins: round r covers bins in alternating order
    nodes_per_bin = P
    perm = np.empty(cfg.N, dtype=np.int64)
    fwd = np.arange(nbins)
    bwd = fwd[::-1]
    pos_in_bin = np.zeros(nbins, dtype=np.int64)
    idx = 0
    r = 0
    npts = len(order)
    while idx < npts:
        bins = fwd if (r % 2 == 0) else bwd
        take = min(nbins, npts - idx)
        sel = order[idx:idx + take]
        b = bins[:take]
        # new id: bin b -> core = b // NB, block = b % NB
        core = b // NB
        blk = b % NB
        perm[sel] = core * SH + blk * P + pos_in_bin[b]
        pos_in_bin[b] += 1
        idx += take
        r += 1
    assert pos_in_bin.max() <= nodes_per_bin
    return perm


def _refine_perm(perm, src0, dst0, cfg):
    """Re-bin nodes within each (core, quarter) to equalize the per
    (core, dst-block, src-quarter) edge-cell counts, minimizing the padded
    chunk count sum(ceil(max_core(cell)/128)).

    Quarter membership (and hence every edge's src-quarter) is invariant
    under these moves, so cell profiles can be computed once.
    """
    SH, NB, NPAD = cfg.SH, cfg.NB, cfg.NPAD
    qlo = cfg.QLO
    qblk = [int(q) // P for q in qlo]          # block index at quarter starts
    src = perm[src0]
    dst = perm[dst0]
    # per-node in-profile over src quarters (invariant)
    sq = np.searchsorted(qlo[1:-1], src % SH, side="right")
    pin = np.zeros((NPAD, NQ), dtype=np.int64)
    np.add.at(pin, (dst, sq), 1)

    new_perm_pos = np.arange(NPAD, dtype=np.int64)  # new position per new id

    def pack(order_ids, prof, nblk, caps):
        """Greedy: place nodes (given order) into nblk blocks, cap P nodes
        each, minimizing hinge over caps [nblk, NQ]."""
        fill = np.zeros((nblk, NQ), dtype=np.int64)
        cnt = np.zeros(nblk, dtype=np.int64)
        assign = np.empty(len(order_ids), dtype=np.int64)
        for i, v in enumerate(order_ids):
            p = prof[i]
            over = np.maximum(fill + p - caps, 0) - np.maximum(fill - caps, 0)
            score = over.sum(axis=1).astype(np.float64)
            # tie-break: prefer emptier blocks (balance node counts)
            score += cnt * 1e-6
            score[cnt >= P] = np.inf
            b = int(np.argmin(score))
            assign[i] = b
            fill[b] += p
            cnt[b] += 1
        return assign, fill

    # two rounds: first against the (5,4,4,4)-chunk grid matched to the
    # skewed quarter sizes, then against the chunk grid actually paid for
    # (max over cores), letting overflow consolidate into paid cells.
    caps_all = np.tile(np.array([[4, 4, 4, 5]], dtype=np.int64) * P, (NB, 1))
    for rnd in range(2):
        fills = np.zeros((NCORES, NB, NQ), dtype=np.int64)
        for c in range(NCORES):
            for Q in range(NQ):
                blo, bhi = qblk[Q], qblk[Q + 1]
                ids = np.arange(c * SH + qlo[Q], c * SH + qlo[Q + 1])
                prof = pin[ids]
                o = np.argsort(-prof.sum(axis=1), kind="stable")
                ids, prof = ids[o], prof[o]
                assign, fill = pack(ids, prof, bhi - blo, caps_all[blo:bhi])
                fills[c, blo:bhi] = fill
                # positions: stable order within block
                order2 = np.argsort(assign, kind="stable")
                srt = assign[order2]
                startb = np.searchsorted(srt, np.arange(bhi - blo))
                posn = c * SH + (blo + srt) * P + (np.arange(len(ids)) -
                                                   startb[srt])
                new_perm_pos[ids[order2]] = posn
        caps_all = np.ceil(fills.max(axis=0) / P).astype(np.int64) * P
    # compose: old id -> phase1 new id -> refined position
    return new_perm_pos[perm]


def _plan(src, dst, cfg):
    """Group (non-self) edges by (core, dst-block, src-chunk).

    src/dst are NEW (relabeled) node ids. Returns the uniform chunk plan.
    """
    SH, NB = cfg.SH, cfg.NB
    c = dst // SH
    dloc = dst - c * SH
    b = dloc // P
    dl = dloc - b * P
    # src chunk + index within chunk segment
    sc = src % SH
    q = np.searchsorted(cfg.QLO[1:-1], sc, side="right")
    iseg = (src // SH) * np.asarray(cfg.QROWS)[q] + (sc - cfg.QLO[q])
    key = ((c * NB + b) * NQ + q).astype(np.int64)
    counts = np.bincount(key, minlength=NCORES * NB * NQ).reshape(NCORES, NB, NQ)
    order = np.argsort(key, kind="stable")
    starts = np.zeros(NCORES * NB * NQ + 1, dtype=np.int64)
    np.cumsum(counts.reshape(-1), out=starts[1:])
    nch = np.ceil(counts.max(axis=0) / P).astype(np.int64)  # [NB, NQ] uniform
    sbs = [list(range(i, min(i + cfg.SBSZ, NB))) for i in range(0, NB, cfg.SBSZ)]
    return {
        "order": order, "starts": starts, "counts": counts,
        "nch": nch, "sbs": sbs, "iseg": iseg, "dl": dl,
    }


def _core_arrays(plan, core, cfg):
    """Build idx (gather stream, (sb,q,b) order) + dstl ((wave,q,b)-major)."""
    nch, sbs = plan["nch"], plan["sbs"]
    order, starts = plan["order"], plan["starts"]
    iseg, dl = plan["iseg"], plan["dl"]
    NB = cfg.NB

    cell_iv = {}
    cell_dv = {}
    for b in range(NB):
        for q in range(NQ):
            n_ch = nch[b][q]
            if n_ch == 0:
                continue
            k = (core * NB + b) * NQ + q
            sl = order[starts[k]:starts[k + 1]]
            # ascending source rows within the cell: consecutive gather
            # descriptors walk the segment in address order (HBM page
            # locality for the latency-bound random reads)
            sl = sl[np.argsort(iseg[sl], kind="stable")]
            pad = n_ch * P - len(sl)
            cell_iv[(b, q)] = np.concatenate([iseg[sl], np.zeros(pad, np.int64)])
            cell_dv[(b, q)] = np.concatenate([dl[sl], np.full(pad, -1, np.int64)])

    idx_cols = []   # per (sb,q): [16, gn*8] int16 segments
    for sb in sbs:
        for q in range(NQ):
            vals = [cell_iv[(b, q)] for b in sb if (b, q) in cell_iv]
            if vals:
                v = np.concatenate(vals)
                idx_cols.append(v.reshape(-1, 16).T.astype(np.int16))
    idx1 = np.tile(np.concatenate(idx_cols, axis=1), (8, 1)) if idx_cols else \
        np.zeros((128, 0), np.int16)

    # dstl columns block-major (per block: its 4 quarters' cells in order)
    dstl_parts = []
    for b in range(NB):
        for q in range(NQ):
            if (b, q) in cell_dv:
                dstl_parts.append(cell_dv[(b, q)].reshape(-1, P).T)
    dstl = np.concatenate(dstl_parts, axis=1).astype(np.float32)
    return np.ascontiguousarray(idx1), \
        np.ascontiguousarray(dstl.astype(ml_dtypes.bfloat16))


def _build_program(plan, cfg):
    SH, NB = cfg.SH, cfg.NB
    IN_DIM, SBSZ, MSPAN = cfg.IN_DIM, cfg.SBSZ, cfg.MSPAN
    KC = IN_DIM // P
    nch, sbs = plan["nch"], plan["sbs"]
    nchb = nch.sum(axis=1)                      # chunks per block
    totch = int(nchb.sum())
    nchb_max = int(nchb.max())
    # gather-stream offsets per (sbi, q) and per-block offsets within groups
    goff = {}
    boff = {}
    off = 0
    for sbi, sb in enumerate(sbs):
        for q in range(NQ):
            gn = int(sum(nch[b][q] for b in sb))
            goff[(sbi, q)] = (off, gn)
            o = 0
            for b in sb:
                boff[(b, q)] = o
                o += int(nch[b][q])
            off += gn
    gn_max = max(gn for (_, gn) in goff.values())
    doff = np.zeros(NB + 1, dtype=np.int64)
    np.cumsum(nchb, out=doff[1:])
    nchb_sb = [int(sum(nchb[b] for b in sb)) for sb in sbs]
    nchb_sb_max = max(nchb_sb)

    nc = bacc.Bacc("TRN2", target_bir_lowering=False, debug=False,
                   num_devices=NCORES, num_swdge_queues=min(4, max(1, cfg.QROT)))
    t_xT = nc.declare_dram_parameter("xT", [IN_DIM, SH], BF16, isOutput=False)
    t_W1 = nc.declare_dram_parameter("W1", [IN_DIM, F1], BF16, isOutput=False)
    t_W2 = nc.declare_dram_parameter("W2", [F1, F2], BF16, isOutput=False)
    t_b1b = nc.declare_dram_parameter("b1b", [P, F1], F32, isOutput=False)
    t_b2b = nc.declare_dram_parameter("b2b", [P, F2], F32, isOutput=False)
    t_degc = nc.declare_dram_parameter("degc", [P, NB], F32, isOutput=False)
    t_degr = (None if cfg.ZB2 else
              nc.declare_dram_parameter("degr", [NB * P], F32, isOutput=False))
    t_idx = nc.declare_dram_parameter("idx", [P, totch * 8], I16, isOutput=False)
    t_dstl = nc.declare_dram_parameter("dstl", [P, totch], BF16, isOutput=False)
    t_y = nc.declare_dram_parameter("y", [SH, F2], F32, isOutput=True)

    # Local (non-Shared) collective outputs: dma_gather reads from the
    # Shared scratchpad run ~28% slower per descriptor (~+1ms over the
    # kernel), and the collectives have plenty of slack to take the
    # non-Shared path instead.
    _aspace = "Local" if cfg.LOCALTAB else "Shared"
    tab1_fq = [nc.dram_tensor(f"tab1_full{q}", [cfg.SEGSZ[q], F1], BF16,
                              addr_space=_aspace) for q in range(NQ)]
    tab2_fq = [nc.dram_tensor(f"tab2_full{q}", [cfg.SEGSZ[q], F1], BF16,
                              addr_space=_aspace) for q in range(NQ)]

    with tile.TileContext(nc) as tc:
        with (
            tc.tile_pool(name="dram", bufs=1, space="DRAM") as dram,
            tc.tile_pool(name="consts", bufs=1) as consts,
            tc.tile_pool(name="sb", bufs=3) as pool,
            tc.tile_pool(name="stage", bufs=2) as stage,
            tc.tile_pool(name="psum", bufs=2, space="PSUM") as psum,
        ):
            tab1_shard = dram.tile([SH, F1], BF16)
            tab2_shard = dram.tile([SH, F1], BF16)

            # ---- constants
            iota_i = consts.tile([P, P], I32)
            nc.gpsimd.iota(iota_i[:], pattern=[[1, P]], base=0, channel_multiplier=0)
            iota_p = consts.tile([P, P], I32)
            nc.gpsimd.iota(iota_p[:], pattern=[[0, P]], base=0, channel_multiplier=1)
            iota_bf = consts.tile([P, P], BF16)
            nc.vector.tensor_copy(iota_bf[:], iota_i[:])
            ident = consts.tile([P, P], BF16)
            nc.vector.tensor_tensor(out=ident[:], in0=iota_i[:], in1=iota_p[:],
                                    op=mybir.AluOpType.is_equal)
            iota_rep = consts.tile([P, P, nchb_max], BF16)
            nc.vector.tensor_copy(
                iota_rep[:],
                iota_bf[:, :, None].to_broadcast([P, P, nchb_max]))

            W1_sb = consts.tile([P, KC, F1], BF16)
            nc.sync.dma_start(out=W1_sb[:],
                              in_=t_W1[:].rearrange("(c p) f -> p c f", p=P))
            W2_bf = consts.tile([P, F2], BF16)
            nc.sync.dma_start(out=W2_bf[:], in_=t_W2[:])
            b1b = consts.tile([P, F1], F32)
            nc.sync.dma_start(out=b1b[:], in_=t_b1b[:])
            b2b = consts.tile([P, F2], F32)
            nc.sync.dma_start(out=b2b[:], in_=t_b2b[:])

            degc = consts.tile([P, NB], F32)
            nc.sync.dma_start(out=degc[:], in_=t_degc[:])
            sq = consts.tile([P, NB], F32)
            nc.scalar.sqrt(sq[:], degc[:])
            dinvc = consts.tile([P, NB], F32)
            nc.vector.reciprocal(dinvc[:], sq[:])
            # dinv^2 per (node, block): relu(dinv*agg)*dinv == relu(dinv2*agg)
            dinv2c = consts.tile([P, NB], F32)
            nc.vector.reciprocal(dinv2c[:], degc[:])

            dinvb = None
            if not cfg.ZB2:
                dinvb = consts.tile([P, NB * P], BF16)
                DSPAN = 1568
                for dspan in range(0, NB * P, DSPAN):
                    dw = min(DSPAN, NB * P - dspan)
                    degb_t = pool.tile([P, DSPAN], F32, tag="degb")
                    nc.sync.dma_start(
                        out=degb_t[:, :dw],
                        in_=t_degr[None, dspan:dspan + dw].to_broadcast([P, dw]))
                    sqb_t = pool.tile([P, DSPAN], F32, tag="sqb")
                    nc.scalar.sqrt(sqb_t[:, :dw], degb_t[:, :dw])
                    rec_t = pool.tile([P, DSPAN], F32, tag="recb")
                    nc.vector.reciprocal(rec_t[:, :dw], sqb_t[:, :dw])
                    nc.vector.tensor_copy(dinvb[:, dspan:dspan + dw], rec_t[:, :dw])

            # ---- S0: h1 = x @ W1 (node-major), p1 = dinv*h1 -> tab1_shard
            # quarter-by-quarter; AllGather each quarter as soon as written.
            for q in range(NQ):
                qlo, qhi = int(cfg.QLO[q]), int(cfg.QLO[q + 1])
                for s0 in range(qlo, qhi, MSPAN):
                    mw = min(MSPAN, qhi - s0)
                    nsub = mw // P
                    xt = pool.tile([P, KC, MSPAN], BF16, tag="xT", bufs=2)
                    nc.sync.dma_start(
                        out=xt[:, :, :mw],
                        in_=t_xT[:, s0:s0 + mw].rearrange("(c p) m -> p c m", p=P))
                    p1s = stage.tile([P, MSPAN // P, F1], BF16, tag="p1s")
                    for sub in range(nsub):
                        moff = sub * P
                        hps = psum.tile([P, F1], F32, tag="aux")
                        for kc in range(KC):
                            nc.tensor.matmul(
                                out=hps[:],
                                lhsT=xt[:, kc, moff:moff + P],
                                rhs=W1_sb[:, kc, :],
                                start=(kc == 0), stop=(kc == KC - 1))
                        B = (s0 + moff) // P
                        nc.scalar.mul(p1s[:, sub, :], hps[:], dinvc[:, B:B + 1])
                    nc.sync.dma_start(
                        out=tab1_shard[s0:s0 + mw, :].rearrange(
                            "(c p) f -> p c f", p=P),
                        in_=p1s[:, :nsub, :])
                # AG1 dispatch deferred into the L1 pass (quarter-major head
                # interleave): see ag1_fire below.

            # ---- aggregation pass (shared for L1/L2)
            qctr = [0]  # global gather-queue rotation (balanced across groups)

            def agg_pass(layer, tabq, tab_shard, out_cb, ag_fire=None,
                         nhead=3):
                mtiles_all = {}

                def emit_gathers(sbi, q):
                    off, gn = goff[(sbi, q)]
                    if gn == 0:
                        return
                    idxt = pool.tile([P, gn_max * 8], I16, tag="idx",
                                     bufs=14)
                    nc.scalar.dma_start(
                        out=idxt[:, :gn * 8],
                        in_=t_idx[:, off * 8:(off + gn) * 8])
                    # deep-buffer the gather stream: with only ~1 group
                    # of tiles the gathers inherit every downstream
                    # bubble (isolated gather rate is ~3x the in-kernel
                    # rate at bufs=5)
                    mt = pool.tile([P, gn_max, F1], BF16, tag="mq",
                                   bufs=14)
                    if cfg.NOGATHER:
                        nc.gpsimd.memset(mt[:, :gn, :], 0.5)
                    else:
                        # HW wedges above 1024 idxs/call (65 ring
                        # entries); cap chunks per call
                        GMAX = cfg.GMAX
                        for g0 in range(0, gn, GMAX):
                            gw = min(GMAX, gn - g0)
                            nc.gpsimd.dma_gather(
                                out_ap=mt[:, g0:g0 + gw, :],
                                in_ap=tabq[q][:],
                                idxs_ap=idxt[:, g0 * 8:(g0 + gw) * 8],
                                num_idxs=gw * P, num_idxs_reg=gw * P,
                                elem_size=F1,
                                queue_num=qctr[0] % cfg.QROT)
                            qctr[0] += 1
                    mtiles_all[(sbi, q)] = mt

                # head: interleave the first nhead groups' gathers
                # quarter-major with the AG dispatches, so the Pool stream
                # never queues a gather behind a later quarter's AG
                # input-wait
                nh = min(nhead, len(sbs)) if ag_fire is not None else 0
                for q in range(NQ):
                    if ag_fire is not None:
                        ag_fire(q)
                    for sbi in range(nh):
                        emit_gathers(sbi, q)

                for sbi, sb in enumerate(sbs):
                    if sbi >= nh:
                        for q in range(NQ):
                            emit_gathers(sbi, q)
                    mtiles = {q: mtiles_all.pop((sbi, q))
                              for q in range(NQ) if (sbi, q) in mtiles_all}
                    dsb = pool.tile([P, nchb_sb_max], BF16, tag="dstl")
                    d0 = int(doff[sb[0]])
                    nsb = nchb_sb[sbi]
                    nc.scalar.dma_start(
                        out=dsb[:, :nsb], in_=t_dstl[:, d0:d0 + nsb])
                    # local table rows for the group's blocks (self-loop term)
                    tbsb = pool.tile([P, SBSZ, F1], BF16, tag="tblk", bufs=2)
                    nc.sync.dma_start(
                        out=tbsb[:, :len(sb), :],
                        in_=tab_shard[sb[0] * P:(sb[-1] + 1) * P, :].rearrange(
                            "(c p) f -> p c f", p=P))
                    if cfg.NOMM:
                        continue
                    for b in sb:
                        nb_ch = int(nchb[b])
                        lo = int(doff[b]) - d0
                        tblk = tbsb[:, b - sb[0], :]
                        oh = pool.tile([P, P, nchb_max], BF16, tag="oh")
                        if cfg.NOONEHOT:
                            nc.vector.memset(oh[:, :, :nb_ch], 0.001)
                        else:
                            # all-packed APs -> DVE 2x mode
                            nc.vector.tensor_tensor(
                                out=oh[:, :, :nb_ch],
                                in0=dsb[:, None, lo:lo + nb_ch].to_broadcast(
                                    [P, P, nb_ch]),
                                in1=iota_rep[:, :, :nb_ch],
                                op=mybir.AluOpType.is_equal)
                        agg = psum.tile([P, P], F32, tag="agg",
                                        bufs=cfg.PSB)
                        j = 0
                        for q in range(NQ):
                            for i in range(int(nch[b][q])):
                                m = mtiles[q][:, boff[(b, q)] + i, :]
                                o = oh[:, :, j]
                                if layer == 1:
                                    nc.tensor.matmul(
                                        out=agg[:], lhsT=o, rhs=m,
                                        start=(j == 0), stop=False)
                                else:
                                    nc.tensor.matmul(
                                        out=agg[:], lhsT=m, rhs=o,
                                        start=(j == 0), stop=False)
                                j += 1
                        # self-loop: agg += I^T @ tblk (node-major) or
                        # tblk^T @ I (feature-major)
                        if layer == 1:
                            nc.tensor.matmul(
                                out=agg[:], lhsT=ident[:], rhs=tblk[:],
                                start=(nb_ch == 0), stop=True)
                        else:
                            nc.tensor.matmul(
                                out=agg[:], lhsT=tblk[:], rhs=ident[:],
                                start=(nb_ch == 0), stop=True)
                        out_cb(b, agg)

            # ---- L1: node-major agg; epilogue -> tab2_shard (+ chunked AG2)
            l1_stage = {}
            qmark = np.searchsorted(cfg.QLO[1:], (np.arange(NB) + 1) * P)
            qend = set(int(q) // P - 1 for q in cfg.QLO[1:])
            # issue each AG2 chunk one group after its quarter completes so
            # the t2 flush has landed (epilogues are on ACT, so the wait is
            # short)
            ag2_at = {}
            for qq, qe in enumerate(sorted(qend)):
                ag2_at.setdefault(min(qe + SBSZ, NB - 1), []).append(qq)

            def ag1_fire(q):
                if cfg.SKIPAG:
                    return
                qlo, qhi = int(cfg.QLO[q]), int(cfg.QLO[q + 1])
                nc.gpsimd.collective_compute(
                    "AllGather", mybir.AluOpType.bypass,
                    ins=[tab1_shard[qlo:qhi, :].opt()],
                    outs=[tab1_fq[q][:].opt()],
                    replica_groups=[list(range(NCORES))])
            l1_plo = [0]  # first block not yet flushed to DRAM

            def l1_out(b, agg):
                if b % SBSZ == 0:
                    l1_stage[b // SBSZ] = stage.tile([P, SBSZ, F1], BF16,
                                                     tag="t2", name="t2")
                t2 = l1_stage[b // SBSZ]
                if cfg.ZB1:
                    # t2 = relu(dinv*agg + 0)*dinv = relu(dinv2*agg); one ACT
                    nc.scalar.activation(
                        t2[:, b % SBSZ, :], agg[:],
                        mybir.ActivationFunctionType.Relu,
                        scale=dinv2c[:, b:b + 1])
                else:
                    v = pool.tile([P, F1], F32, tag="v")
                    nc.vector.scalar_tensor_tensor(
                        out=v[:], in0=agg[:], scalar=dinvc[:, b:b + 1],
                        in1=b1b[:], op0=mybir.AluOpType.mult,
                        op1=mybir.AluOpType.add)
                    r = pool.tile([P, F1], F32, tag="r")
                    nc.scalar.activation(r[:], v[:],
                                         mybir.ActivationFunctionType.Relu)
                    nc.vector.tensor_scalar_mul(t2[:, b % SBSZ, :], r[:],
                                                dinvc[:, b:b + 1])
                if b % SBSZ == SBSZ - 1 or b == NB - 1 or b in qend:
                    plo = l1_plo[0]
                    nfb = b - plo + 1
                    nc.sync.dma_start(
                        out=tab2_shard[plo * P:(plo + nfb) * P, :].rearrange(
                            "(c p) f -> p c f", p=P),
                        in_=t2[:, plo % SBSZ:plo % SBSZ + nfb, :])
                    l1_plo[0] = b + 1
                # fire pending AG2 chunks (delayed past their quarter end)
                if cfg.PHASES >= 2 and not cfg.SKIPAG:
                    for q in ag2_at.get(b, []):
                        qlo, qhi = int(cfg.QLO[q]), int(cfg.QLO[q + 1])
                        nc.gpsimd.collective_compute(
                            "AllGather", mybir.AluOpType.bypass,
                            ins=[tab2_shard[qlo:qhi, :].opt()],
                            outs=[tab2_fq[q][:].opt()],
                            replica_groups=[list(range(NCORES))])

            if cfg.PHASES >= 1:
                agg_pass(1, tab1_fq, tab1_shard, l1_out, ag_fire=ag1_fire)

            # ---- L2: feature-major agg; epilogue -> y
            l2_stage = {}

            def l2_out(b, agg):
                w = pool.tile([P, P], BF16, tag="w")
                if cfg.ZB2:
                    # defer the dinv_d scale to after the W2 matmul (it is a
                    # per-dst scalar); PSUM->SBUF cast rides the ACT copy
                    nc.scalar.copy(w[:], agg[:])
                else:
                    nc.vector.tensor_tensor(
                        out=w[:], in0=agg[:], in1=dinvb[:, b * P:(b + 1) * P],
                        op=mybir.AluOpType.mult)
                o2f = psum.tile([P, F1], F32, tag="aux")
                o2 = o2f[:, :F2]
                nc.tensor.matmul(out=o2, lhsT=w[:], rhs=W2_bf[:],
                                 start=True, stop=True)
                if b % SBSZ == 0:
                    l2_stage[b // SBSZ] = stage.tile([P, SBSZ, F2], F32,
                                                     tag="ys", name="ys")
                ys = l2_stage[b // SBSZ]
                if cfg.ZB2:
                    nc.scalar.mul(ys[:, b % SBSZ, :], o2[:],
                                  dinvc[:, b:b + 1])
                else:
                    nc.vector.tensor_add(ys[:, b % SBSZ, :], o2[:], b2b[:])
                if b % SBSZ == SBSZ - 1 or b == NB - 1:
                    blo = (b // SBSZ) * SBSZ
                    nfb = b - blo + 1
                    nc.scalar.dma_start(
                        out=t_y[blo * P:(blo + nfb) * P, :].rearrange(
                            "(c p) f -> p c f", p=P),
                        in_=ys[:, :nfb, :])

            if cfg.PHASES >= 2:
                agg_pass(2, tab2_fq, tab2_shard, l2_out)
            if cfg.NOMM:
                zt = pool.tile([P, F2], F32, tag="dbg0")
                nc.vector.memset(zt[:], 0.0)
                for bb in range(0, SH, P):
                    nc.scalar.dma_start(out=t_y[bb:bb + P, :], in_=zt[:])
            if cfg.PHASES < 2:
                # debug exit: y <- copy of tab1_full0 head
                dbt = pool.tile([P, F2], BF16, tag="dbgb")
                nc.sync.dma_start(out=dbt[:], in_=tab1_fq[0][0:P, 0:F2])
                dbg = pool.tile([P, F2], F32, tag="dbg")
                nc.vector.tensor_copy(dbg[:], dbt[:])
                for bb in range(0, SH, P):
                    nc.scalar.dma_start(out=t_y[bb:bb + P, :], in_=dbg[:])

    nc.compile()
    return nc


def _prep(x, edge_index, W1, b1, W2, b2, cfg=DEFAULT_CFG):
    N, SH, NB, NPAD = cfg.N, cfg.SH, cfg.NB, cfg.NPAD
    src0 = np.asarray(edge_index[0], dtype=np.int64)
    dst0 = np.asarray(edge_index[1], dtype=np.int64)
    # degree includes the self loop (matches reference)
    deg0 = (np.bincount(dst0, minlength=N) + 1).astype(np.float32)
    perm = _balanced_perm(deg0, cfg)
    perm = _refine_perm(perm, src0, dst0, cfg)
    src = perm[src0]
    dst = perm[dst0]
    degp = np.ones(NPAD, dtype=np.float32)  # pad nodes: deg 1 (no edges)
    degp[perm] = deg0
    plan = _plan(src, dst, cfg)

    xb = np.asarray(x, dtype=np.float32)
    W1b = np.asarray(W1, dtype=ml_dtypes.bfloat16)
    W2b = np.asarray(W2, dtype=ml_dtypes.bfloat16)
    b1b = np.ascontiguousarray(np.tile(np.asarray(b1, np.float32)[None, :], (P, 1)))
    b2b = np.ascontiguousarray(np.tile(np.asarray(b2, np.float32)[None, :], (P, 1)))

    # x rows in new order: xp[newid] = x[oldid]
    invp = np.empty(NPAD, dtype=np.int64)
    invp.fill(0)
    invp[perm] = np.arange(N)
    has = np.zeros(NPAD, dtype=bool)
    has[perm] = True

    in_maps = []
    for c in range(NCORES):
        rows = invp[c * SH:(c + 1) * SH]
        mask = has[c * SH:(c + 1) * SH]
        xs = xb[rows] * mask[:, None]
        xT = np.ascontiguousarray(xs.T.astype(ml_dtypes.bfloat16))
        degsh = degp[c * SH:(c + 1) * SH]
        degc = np.ascontiguousarray(degsh.reshape(NB, P).T)
        idx1, dstl = _core_arrays(plan, c, cfg)
        in_maps.append({
            "xT": xT, "W1": W1b, "W2": W2b, "b1b": b1b, "b2b": b2b,
            "degc": degc, "degr": degsh, "idx": idx1, "dstl": dstl,
        })
    return plan, in_maps, perm


def _get_program(plan, cfg=DEFAULT_CFG):
    key = (hashlib.sha256(plan["nch"].tobytes()).hexdigest() +
           f"{cfg.N}_{cfg.PHASES}_{cfg.NOGATHER}_{cfg.NOONEHOT}_{cfg.SKIPAG}_{cfg.GMAX}_{cfg.QROT}_{cfg.SBSZ}_{cfg.PSB}_{cfg.NOMM}_{cfg.LOCALTAB}_{cfg.ZB1}_{cfg.ZB2}_{cfg.WG}_v4")
    if key not in _cache:
        _cache[key] = _build_program(plan, cfg)
    return _cache[key]


def _make_runner(nc, cfg):
    """Persistent jitted SPMD executor (mirrors bass2jax.run_bass_via_pjrt's
    multi-core path) so repeated calls reuse the compiled NEFF."""
    import jax
    from jax.sharding import Mesh, PartitionSpec
    from jax.experimental.shard_map import shard_map
    from concourse import bass2jax as b2j

    b2j.install_neuronx_cc_hook()
    assert nc.dbg_addr is None
    partition_name = (nc.partition_id_tensor.name
                      if nc.partition_id_tensor else None)

    in_names, out_names, out_avals = [], [], []
    for alloc in nc.m.functions[0].allocations:
        if not isinstance(alloc, mybir.MemoryLocationSet):
            continue
        name = alloc.memorylocations[0].name
        if alloc.kind == "ExternalInput":
            if name != partition_name:
                in_names.append(name)
        elif alloc.kind == "ExternalOutput":
            out_names.append(name)
            out_avals.append(jax.core.ShapedArray(
                tuple(alloc.tensor_shape), mybir.dt.np(alloc.dtype)))
    n_params = len(in_names)
    n_outs = len(out_names)
    all_names = in_names + out_names
    if partition_name is not None:
        all_names = all_names + [partition_name]
    donate = tuple(range(n_params, n_params + n_outs))

    def _body(*args):
        operands = list(args)
        if partition_name is not None:
            operands.append(b2j.partition_id_tensor())
        outs = b2j._bass_exec_p.bind(
            *operands,
            out_avals=tuple(out_avals),
            in_names=tuple(all_names),
            out_names=tuple(out_names),
            lowering_input_output_aliases=(),
            sim_require_finite=True,
            sim_require_nnan=True,
            nc=nc,
        )
        return tuple(outs)

    devices = jax.devices()[:NCORES]
    mesh = Mesh(np.asarray(devices), ("core",))
    sharded = jax.jit(
        shard_map(_body, mesh=mesh,
                  in_specs=(PartitionSpec("core"),) * (n_params + n_outs),
                  out_specs=(PartitionSpec("core"),) * n_outs,
                  check_rep=False),
        donate_argnums=donate, keep_unused=True)
    return {
        "fn": sharded, "in_names": in_names, "out_names": out_names,
        "out_avals": out_avals, "mesh": mesh,
    }


def _runner_args(runner, in_maps):
    concat_in = [
        np.concatenate([np.asarray(in_maps[c][k]) for c in range(NCORES)], 0)
        for k in runner["in_names"]
    ]
    zeros = [
        np.zeros((NCORES * a.shape[0],) + tuple(a.shape[1:]), a.dtype)
        for a in runner["out_avals"]
    ]
    return concat_in, zeros


def _get_runner(plan, cfg=DEFAULT_CFG):
    key = ("runner_" + hashlib.sha256(plan["nch"].tobytes()).hexdigest() +
           f"{cfg.N}_{cfg.PHASES}_{cfg.NOGATHER}_{cfg.NOONEHOT}_{cfg.SKIPAG}_{cfg.GMAX}_{cfg.QROT}_{cfg.SBSZ}_{cfg.PSB}_{cfg.NOMM}_{cfg.LOCALTAB}_{cfg.ZB1}_{cfg.ZB2}_{cfg.WG}_v4")
    if key not in _cache:
        _cache[key] = _make_runner(_get_program(plan, cfg), cfg)
    return _cache[key]


def _cfg_for(b1, b2):
    zb1 = bool(np.all(np.asarray(b1) == 0))
    zb2 = bool(np.all(np.asarray(b2) == 0))
    if zb1 and zb2:
        return DEFAULT_CFG
    return CFG(N=100000, IN_DIM=512, ZB1=zb1, ZB2=zb2)


def kernel(x, edge_index, W1, b1, W2, b2):
    cfg = _cfg_for(b1, b2)
    plan, in_maps, perm = _prep(x, edge_index, W1, b1, W2, b2, cfg)
    runner = _get_runner(plan, cfg)
    concat_in, zeros = _runner_args(runner, in_maps)
    outs = runner["fn"](*concat_in, *zeros)
    y = np.asarray(outs[runner["out_names"].index("y")]).reshape(cfg.NPAD, F2)
    return np.ascontiguousarray(y[perm])


def _floor_overhead(iters=10):
    """Per-call dispatch floor of this PJRT/axon path (trivial program)."""
    import time
    import jax
    from jax.sharding import NamedSharding, PartitionSpec
    import concourse.tile as tile_mod

    if "floor_nc" not in _cache:
        nc = bacc.Bacc("TRN2", target_bir_lowering=False, debug=False,
                       num_devices=NCORES)
        x = nc.declare_dram_parameter("x", [P, P], F32, isOutput=False)
        y = nc.declare_dram_parameter("y", [P, P], F32, isOutput=True)
        with tile_mod.TileContext(nc) as tc:
            with tc.tile_pool(name="sbuf", bufs=2) as pool:
                t = pool.tile([P, P], F32)
                nc.sync.dma_start(out=t[:], in_=x[:])
                t2 = pool.tile([P, P], F32)
                nc.vector.tensor_scalar_mul(t2[:], t[:], 3.0)
                nc.sync.dma_start(out=y[:], in_=t2[:])
        nc.compile()
        _cache["floor_nc"] = _make_runner(nc, None)
    runner = _cache["floor_nc"]
    in_maps = [{"x": np.zeros((P, P), np.float32)} for _ in range(NCORES)]
    concat_in, zeros = _runner_args(runner, in_maps)
    sh = NamedSharding(runner["mesh"], PartitionSpec("core"))
    dev_in = [jax.device_put(a, sh) for a in concat_in]
    zsets = [[jax.device_put(z, sh) for z in zeros] for _ in range(iters + 1)]
    jax.block_until_ready([dev_in, zsets])
    jax.block_until_ready(runner["fn"](*dev_in, *zsets[0]))
    ts = []
    for i in range(iters):
        t0 = time.perf_counter()
        jax.block_until_ready(runner["fn"](*dev_in, *zsets[i + 1]))
        ts.append(time.perf_counter() - t0)
    return min(ts)


def benchmark(inputs, iters=5):
    """Device-time estimate via pipelined-dispatch slope.

    The axon/PJRT channel latency is large and bimodal, so single-call
    wall-minus-floor is unusable. Instead dispatch n back-to-back
    executions (device runs them contiguously) and fit the slope:
    est = (min T(n_hi) - min T(n_lo)) / (n_hi - n_lo), which cancels the
    per-batch channel overhead.
    """
    import time
    import jax
    from jax.sharding import NamedSharding, PartitionSpec

    cfg = _cfg_for(inputs["b1"], inputs["b2"])
    plan, in_maps, _perm = _prep(**inputs, cfg=cfg)
    runner = _get_runner(plan, cfg)
    concat_in, zeros = _runner_args(runner, in_maps)
    sh = NamedSharding(runner["mesh"], PartitionSpec("core"))
    dev_in = [jax.device_put(a, sh) for a in concat_in]

    def run_n(n):
        zsets = [[jax.device_put(z, sh) for z in zeros] for _ in range(n)]
        jax.block_until_ready(zsets)
        t0 = time.perf_counter()
        outs = [runner["fn"](*dev_in, *zsets[i]) for i in range(n)]
        jax.block_until_ready(outs)
        return time.perf_counter() - t0

    run_n(1)  # warmup (compile)
    n_lo, n_hi = 1, 17
    t_lo, t_hi = [], []
    reps = max(4, (iters + 1) // 2)
    for _ in range(reps):
        t_lo.append(run_n(n_lo))
        t_hi.append(run_n(n_hi))
    est = (min(t_hi) - min(t_lo)) / (n_hi - n_lo)
    floor = min(t_lo)
    raw = min(t_lo)
    return {"raw_ns": int(raw * 1e9), "floor_ns": int(floor * 1e9),
            "est_ns": int(max(est, raw * 0.001) * 1e9)}



# revision 54
# speedup vs baseline: 1.0657x; 1.0528x over previous
"""Two-layer GCN (PyG GCNConv x2 + ReLU) on 8 Trainium2 NeuronCores.

Sharding: nodes are balance-relabeled and partitioned across the 8 cores
(12544 each incl. pad; N padded 100000 -> 100352 = 8*98*128). Each core:
  S0: h1 = x_shard @ W1 (bf16 matmul), p1 = dinv*h1 -> bf16 table shard
      (PSUM->SBUF scale rides an ACT copy); written quarter-by-quarter,
      each quarter AllGathered as soon as ready (4 chunked AllGathers ->
      chunk-major replicated table, overlapping the collective with the S0
      tail + L1 head).
  L1: for each dst-block of 128 nodes, gather p1[src] rows for its incoming
      non-self edges (dma_gather, int16 indices per <=29696-row segment
      aligned with the AG chunks, 1024 idxs/call rotated across 4 SWDGE
      queues - the single-queue descriptor path is the kernel's bottleneck),
      build 0/1 selector tiles on DVE with an all-packed-AP is_equal (2x DVE
      mode), segment-sum via TensorE matmuls in PSUM; the self-loop term is
      added with one identity matmul from the local table block (no gather).
      Epilogue relu(dinv^2*agg) (zero-bias fast path, one ACT op) -> bf16
      table2 shard, AllGathered in 4 quarter chunks that fire mid-L1 and
      overlap with the L1 tail / L2 head.
  L2: same gather/selector pass over table2 (feature-major accumulation),
      then out = dinv*(agg2 @ W2) -> fp32 output shard (dinv applied after
      the W2 matmul as a per-partition ACT scale).
Host reassembles the 8 output shards and inverts the relabeling.

Edges are grouped per (core, dst-block, src-chunk) with chunk counts made
uniform across cores so a single SPMD program serves all 8 cores; padding
slots gather row 0 and carry a -1 dst that the selector maps to zero.
The first nhead=3 groups' gathers are emitted quarter-major, interleaved
with the AG dispatches, so no gather ever queues behind a later quarter's
AG input-wait on the in-order Pool engine. Nonzero b1/b2 fall back to a
general (DVE) epilogue path chosen at program-build time.
"""

import hashlib
import sys

for _p in ("/opt/trn_rl_repo",):
    if _p not in sys.path:
        sys.path.insert(0, _p)

import numpy as np
import ml_dtypes

import concourse.bass as bass  # noqa: F401  (engine types via nc)
import concourse.bacc as bacc
import concourse.mybir as mybir
import concourse.tile as tile

BF16 = mybir.dt.bfloat16
F32 = mybir.dt.float32
I16 = mybir.dt.int16
I32 = mybir.dt.int32

P = 128
NCORES = 8
F1 = 128
F2 = 64
NQ = 4


class CFG:
    def __init__(self, N, IN_DIM, SBSZ=6, MSPAN=1280, PHASES=2,
                 NOGATHER=False, NOONEHOT=False, SKIPAG=False,
                 GMAX=8, QROT=4, PSB=2, NOMM=False, LOCALTAB=True,
                 ZB1=True, ZB2=True, WG=3):
        # WG: sb-groups per wave. Blocks of a wave keep persistent PSUM
        # accumulators across the 4 src-quarters so gather tiles free per
        # quarter (not per 4-quarter group), and AllGathers interleave with
        # wave-0 quarters.
        self.WG = WG
        # ZB1/ZB2: bias vectors known to be all-zero (host-checked) -> the
        # whole epilogue collapses to one ACT op per block (scale folded in).
        self.ZB1 = ZB1
        self.ZB2 = ZB2
        self.PHASES = PHASES
        self.NOGATHER = NOGATHER
        self.NOONEHOT = NOONEHOT
        self.SKIPAG = SKIPAG
        self.GMAX = GMAX
        self.QROT = QROT
        self.PSB = PSB
        self.NOMM = NOMM
        self.LOCALTAB = LOCALTAB
        self.N = N
        self.NPAD = NCORES * 98 * P            # 100352
        self.SH = self.NPAD // NCORES          # 12544
        self.NB = self.SH // P                 # 98
        # per-shard quarter row counts (block-aligned), sum = SH. The last
        # quarter is oversized so its edge cells target ~604 of a 640
        # (5-chunk) cap while the others target ~479 of 512 (4 chunks) --
        # this gives the balance packer ~1.3 sigma of slack per cell.
        # Small quarters FIRST: AG chunk 0 of each table covers fewer rows,
        # completes sooner, and unblocks the L1/L2 gather streams earlier.
        self.QROWS = [2944, 2944, 2944, 3712]
        self.QLO = np.concatenate([[0], np.cumsum(self.QROWS)]).astype(np.int64)
        # per-chunk full-table segment sizes (8 * qrows), all < 32768
        self.SEGSZ = [NCORES * r for r in self.QROWS]
        self.IN_DIM = IN_DIM
        self.SBSZ = SBSZ
        self.MSPAN = MSPAN


DEFAULT_CFG = CFG(N=100000, IN_DIM=512)

_cache = {}


def _balanced_perm(deg, cfg):
    """Relabel nodes so per-(core,block) in-degree sums are balanced.

    Returns perm: old node id -> new node id in [0, NPAD).
    New id layout: core c owns [c*SH, (c+1)*SH); block b of core c is
    rows [c*SH + b*P, c*SH + (b+1)*P).
    """
    NPAD, SH, NB = cfg.NPAD, cfg.SH, cfg.NB
    nbins = NCORES * NB
    order = np.argsort(-deg, kind="stable")  # heavy nodes first
    # snake-deal node ranks into bins: round r covers bins in alternating order
    nodes_per_bin = P
    perm = np.empty(cfg.N, dtype=np.int64)
    fwd = np.arange(nbins)
    bwd = fwd[::-1]
    pos_in_bin = np.zeros(nbins, dtype=np.int64)
    idx = 0
    r = 0
    npts = len(order)
    while idx < npts:
        bins = fwd if (r % 2 == 0) else bwd
        take = min(nbins, npts - idx)
        sel = order[idx:idx + take]
        b = bins[:take]
        # new id: bin b -> core = b // NB, block = b % NB
        core = b // NB
        blk = b % NB
        perm[sel] = core * SH + blk * P + pos_in_bin[b]
        pos_in_bin[b] += 1
        idx += take
        r += 1
    assert pos_in_bin.max() <= nodes_per_bin
    return perm


def _refine_perm(perm, src0, dst0, cfg):
    """Re-bin nodes within each (core, quarter) to equalize the per
    (core, dst-block, src-quarter) edge-cell counts, minimizing the padded
    chunk count sum(ceil(max_core(cell)/128)).

    Quarter membership (and hence every edge's src-quarter) is invariant
    under these moves, so cell profiles can be computed once.
    """
    SH, NB, NPAD = cfg.SH, cfg.NB, cfg.NPAD
    qlo = cfg.QLO
    qblk = [int(q) // P for q in qlo]          # block index at quarter starts
    src = perm[src0]
    dst = perm[dst0]
    # per-node in-profile over src quarters (invariant)
    sq = np.searchsorted(qlo[1:-1], src % SH, side="right")
    pin = np.zeros((NPAD, NQ), dtype=np.int64)
    np.add.at(pin, (dst, sq), 1)

    new_perm_pos = np.arange(NPAD, dtype=np.int64)  # new position per new id

    def pack(order_ids, prof, nblk, caps):
        """Greedy: place nodes (given order) into nblk blocks, cap P nodes
        each, minimizing hinge over caps [nblk, NQ]."""
        fill = np.zeros((nblk, NQ), dtype=np.int64)
        cnt = np.zeros(nblk, dtype=np.int64)
        assign = np.empty(len(order_ids), dtype=np.int64)
        for i, v in enumerate(order_ids):
            p = prof[i]
            over = np.maximum(fill + p - caps, 0) - np.maximum(fill - caps, 0)
            score = over.sum(axis=1).astype(np.float64)
            # tie-break: prefer emptier blocks (balance node counts)
            score += cnt * 1e-6
            score[cnt >= P] = np.inf
            b = int(np.argmin(score))
            assign[i] = b
            fill[b] += p
            cnt[b] += 1
        return assign, fill

    # two rounds: first against the (5,4,4,4)-chunk grid matched to the
    # skewed quarter sizes, then against the chunk grid actually paid for
    # (max over cores), letting overflow consolidate into paid cells.
    caps_all = np.tile(np.array([[4, 4, 4, 5]], dtype=np.int64) * P, (NB, 1))
    for rnd in range(2):
        fills = np.zeros((NCORES, NB, NQ), dtype=np.int64)
        for c in range(NCORES):
            for Q in range(NQ):
                blo, bhi = qblk[Q], qblk[Q + 1]
                ids = np.arange(c * SH + qlo[Q], c * SH + qlo[Q + 1])
                prof = pin[ids]
                o = np.argsort(-prof.sum(axis=1), kind="stable")
                ids, prof = ids[o], prof[o]
                assign, fill = pack(ids, prof, bhi - blo, caps_all[blo:bhi])
                fills[c, blo:bhi] = fill
                # positions: stable order within block
                order2 = np.argsort(assign, kind="stable")
                srt = assign[order2]
                startb = np.searchsorted(srt, np.arange(bhi - blo))
                posn = c * SH + (blo + srt) * P + (np.arange(len(ids)) -
                                                   startb[srt])
                new_perm_pos[ids[order2]] = posn
        caps_all = np.ceil(fills.max(axis=0) / P).astype(np.int64) * P
    # compose: old id -> phase1 new id -> refined position
    return new_perm_pos[perm]


def _plan(src, dst, cfg):
    """Group (non-self) edges by (core, dst-block, src-chunk).

    src/dst are NEW (relabeled) node ids. Returns the uniform chunk plan.
    """
    SH, NB = cfg.SH, cfg.NB
    c = dst // SH
    dloc = dst - c * SH
    b = dloc // P
    dl = dloc - b * P
    # src chunk + index within chunk segment
    sc = src % SH
    q = np.searchsorted(cfg.QLO[1:-1], sc, side="right")
    iseg = (src // SH) * np.asarray(cfg.QROWS)[q] + (sc - cfg.QLO[q])
    key = ((c * NB + b) * NQ + q).astype(np.int64)
    counts = np.bincount(key, minlength=NCORES * NB * NQ).reshape(NCORES, NB, NQ)
    order = np.argsort(key, kind="stable")
    starts = np.zeros(NCORES * NB * NQ + 1, dtype=np.int64)
    np.cumsum(counts.reshape(-1), out=starts[1:])
    nch = np.ceil(counts.max(axis=0) / P).astype(np.int64)  # [NB, NQ] uniform
    sbs = [list(range(i, min(i + cfg.SBSZ, NB))) for i in range(0, NB, cfg.SBSZ)]
    return {
        "order": order, "starts": starts, "counts": counts,
        "nch": nch, "sbs": sbs, "iseg": iseg, "dl": dl,
    }


def _core_arrays(plan, core, cfg):
    """Build idx (gather stream, (sb,q,b) order) + dstl ((wave,q,b)-major)."""
    nch, sbs = plan["nch"], plan["sbs"]
    order, starts = plan["order"], plan["starts"]
    iseg, dl = plan["iseg"], plan["dl"]
    NB = cfg.NB

    cell_iv = {}
    cell_dv = {}
    for b in range(NB):
        for q in range(NQ):
            n_ch = nch[b][q]
            if n_ch == 0:
                continue
            k = (core * NB + b) * NQ + q
            sl = order[starts[k]:starts[k + 1]]
            # ascending source rows within the cell: consecutive gather
            # descriptors walk the segment in address order (HBM page
            # locality for the latency-bound random reads)
            sl = sl[np.argsort(iseg[sl], kind="stable")]
            pad = n_ch * P - len(sl)
            cell_iv[(b, q)] = np.concatenate([iseg[sl], np.zeros(pad, np.int64)])
            cell_dv[(b, q)] = np.concatenate([dl[sl], np.full(pad, -1, np.int64)])

    idx_cols = []   # per (sb,q): [16, gn*8] int16 segments
    for sb in sbs:
        for q in range(NQ):
            vals = [cell_iv[(b, q)] for b in sb if (b, q) in cell_iv]
            if vals:
                v = np.concatenate(vals)
                idx_cols.append(v.reshape(-1, 16).T.astype(np.int16))
    idx1 = np.tile(np.concatenate(idx_cols, axis=1), (8, 1)) if idx_cols else \
        np.zeros((128, 0), np.int16)

    # dstl columns block-major (per block: its 4 quarters' cells in order)
    dstl_parts = []
    for b in range(NB):
        for q in range(NQ):
            if (b, q) in cell_dv:
                dstl_parts.append(cell_dv[(b, q)].reshape(-1, P).T)
    dstl = np.concatenate(dstl_parts, axis=1).astype(np.float32)
    return np.ascontiguousarray(idx1), \
        np.ascontiguousarray(dstl.astype(ml_dtypes.bfloat16))


def _build_program(plan, cfg):
    SH, NB = cfg.SH, cfg.NB
    IN_DIM, SBSZ, MSPAN = cfg.IN_DIM, cfg.SBSZ, cfg.MSPAN
    KC = IN_DIM // P
    nch, sbs = plan["nch"], plan["sbs"]
    nchb = nch.sum(axis=1)                      # chunks per block
    totch = int(nchb.sum())
    nchb_max = int(nchb.max())
    # gather-stream offsets per (sbi, q) and per-block offsets within groups
    goff = {}
    boff = {}
    off = 0
    for sbi, sb in enumerate(sbs):
        for q in range(NQ):
            gn = int(sum(nch[b][q] for b in sb))
            goff[(sbi, q)] = (off, gn)
            o = 0
            for b in sb:
                boff[(b, q)] = o
                o += int(nch[b][q])
            off += gn
    gn_max = max(gn for (_, gn) in goff.values())
    doff = np.zeros(NB + 1, dtype=np.int64)
    np.cumsum(nchb, out=doff[1:])
    nchb_sb = [int(sum(nchb[b] for b in sb)) for sb in sbs]
    nchb_sb_max = max(nchb_sb)

    nc = bacc.Bacc("TRN2", target_bir_lowering=False, debug=False,
                   num_devices=NCORES, num_swdge_queues=min(4, max(1, cfg.QROT)))
    t_xT = nc.declare_dram_parameter("xT", [IN_DIM, SH], BF16, isOutput=False)
    t_W1 = nc.declare_dram_parameter("W1", [IN_DIM, F1], BF16, isOutput=False)
    t_W2 = nc.declare_dram_parameter("W2", [F1, F2], BF16, isOutput=False)
    t_b1b = nc.declare_dram_parameter("b1b", [P, F1], F32, isOutput=False)
    t_b2b = nc.declare_dram_parameter("b2b", [P, F2], F32, isOutput=False)
    t_degc = nc.declare_dram_parameter("degc", [P, NB], F32, isOutput=False)
    t_degr = (None if cfg.ZB2 else
              nc.declare_dram_parameter("degr", [NB * P], F32, isOutput=False))
    t_idx = nc.declare_dram_parameter("idx", [P, totch * 8], I16, isOutput=False)
    t_dstl = nc.declare_dram_parameter("dstl", [P, totch], BF16, isOutput=False)
    t_y = nc.declare_dram_parameter("y", [SH, F2], F32, isOutput=True)

    # Local (non-Shared) collective outputs: dma_gather reads from the
    # Shared scratchpad run ~28% slower per descriptor (~+1ms over the
    # kernel), and the collectives have plenty of slack to take the
    # non-Shared path instead.
    _aspace = "Local" if cfg.LOCALTAB else "Shared"
    tab1_fq = [nc.dram_tensor(f"tab1_full{q}", [cfg.SEGSZ[q], F1], BF16,
                              addr_space=_aspace) for q in range(NQ)]
    tab2_fq = [nc.dram_tensor(f"tab2_full{q}", [cfg.SEGSZ[q], F1], BF16,
                              addr_space=_aspace) for q in range(NQ)]

    with tile.TileContext(nc) as tc:
        with (
            tc.tile_pool(name="dram", bufs=1, space="DRAM") as dram,
            tc.tile_pool(name="consts", bufs=1) as consts,
            tc.tile_pool(name="sb", bufs=3) as pool,
            tc.tile_pool(name="stage", bufs=2) as stage,
            tc.tile_pool(name="psum", bufs=2, space="PSUM") as psum,
        ):
            tab1_shard = dram.tile([SH, F1], BF16)
            tab2_shard = dram.tile([SH, F1], BF16)

            # ---- constants
            iota_i = consts.tile([P, P], I32)
            nc.gpsimd.iota(iota_i[:], pattern=[[1, P]], base=0, channel_multiplier=0)
            iota_p = consts.tile([P, P], I32)
            nc.gpsimd.iota(iota_p[:], pattern=[[0, P]], base=0, channel_multiplier=1)
            iota_bf = consts.tile([P, P], BF16)
            nc.vector.tensor_copy(iota_bf[:], iota_i[:])
            ident = consts.tile([P, P], BF16)
            nc.vector.tensor_tensor(out=ident[:], in0=iota_i[:], in1=iota_p[:],
                                    op=mybir.AluOpType.is_equal)
            iota_rep = consts.tile([P, P, nchb_max], BF16)
            nc.vector.tensor_copy(
                iota_rep[:],
                iota_bf[:, :, None].to_broadcast([P, P, nchb_max]))

            W1_sb = consts.tile([P, KC, F1], BF16)
            nc.sync.dma_start(out=W1_sb[:],
                              in_=t_W1[:].rearrange("(c p) f -> p c f", p=P))
            W2_bf = consts.tile([P, F2], BF16)
            nc.sync.dma_start(out=W2_bf[:], in_=t_W2[:])
            b1b = consts.tile([P, F1], F32)
            nc.sync.dma_start(out=b1b[:], in_=t_b1b[:])
            b2b = consts.tile([P, F2], F32)
            nc.sync.dma_start(out=b2b[:], in_=t_b2b[:])

            degc = consts.tile([P, NB], F32)
            nc.sync.dma_start(out=degc[:], in_=t_degc[:])
            sq = consts.tile([P, NB], F32)
            nc.scalar.sqrt(sq[:], degc[:])
            dinvc = consts.tile([P, NB], F32)
            nc.vector.reciprocal(dinvc[:], sq[:])
            # dinv^2 per (node, block): relu(dinv*agg)*dinv == relu(dinv2*agg)
            dinv2c = consts.tile([P, NB], F32)
            nc.vector.reciprocal(dinv2c[:], degc[:])

            dinvb = None
            if not cfg.ZB2:
                dinvb = consts.tile([P, NB * P], BF16)
                DSPAN = 1568
                for dspan in range(0, NB * P, DSPAN):
                    dw = min(DSPAN, NB * P - dspan)
                    degb_t = pool.tile([P, DSPAN], F32, tag="degb")
                    nc.sync.dma_start(
                        out=degb_t[:, :dw],
                        in_=t_degr[None, dspan:dspan + dw].to_broadcast([P, dw]))
                    sqb_t = pool.tile([P, DSPAN], F32, tag="sqb")
                    nc.scalar.sqrt(sqb_t[:, :dw], degb_t[:, :dw])
                    rec_t = pool.tile([P, DSPAN], F32, tag="recb")
                    nc.vector.reciprocal(rec_t[:, :dw], sqb_t[:, :dw])
                    nc.vector.tensor_copy(dinvb[:, dspan:dspan + dw], rec_t[:, :dw])

            # ---- S0: h1 = x @ W1 (node-major), p1 = dinv*h1 -> tab1_shard
            # quarter-by-quarter; AllGather each quarter as soon as written.
            for q in range(NQ):
                qlo, qhi = int(cfg.QLO[q]), int(cfg.QLO[q + 1])
                for s0 in range(qlo, qhi, MSPAN):
                    mw = min(MSPAN, qhi - s0)
                    nsub = mw // P
                    xt = pool.tile([P, KC, MSPAN], BF16, tag="xT", bufs=2)
                    nc.sync.dma_start(
                        out=xt[:, :, :mw],
                        in_=t_xT[:, s0:s0 + mw].rearrange("(c p) m -> p c m", p=P))
                    p1s = stage.tile([P, MSPAN // P, F1], BF16, tag="p1s")
                    for sub in range(nsub):
                        moff = sub * P
                        hps = psum.tile([P, F1], F32, tag="aux")
                        for kc in range(KC):
                            nc.tensor.matmul(
                                out=hps[:],
                                lhsT=xt[:, kc, moff:moff + P],
                                rhs=W1_sb[:, kc, :],
                                start=(kc == 0), stop=(kc == KC - 1))
                        B = (s0 + moff) // P
                        nc.scalar.mul(p1s[:, sub, :], hps[:], dinvc[:, B:B + 1])
                    nc.sync.dma_start(
                        out=tab1_shard[s0:s0 + mw, :].rearrange(
                            "(c p) f -> p c f", p=P),
                        in_=p1s[:, :nsub, :])
                # AG1 dispatch deferred into the L1 pass (quarter-major head
                # interleave): see ag1_fire below.

            # ---- aggregation pass (shared for L1/L2)
            qctr = [0]  # global gather-queue rotation (balanced across groups)

            def agg_pass(layer, tabq, tab_shard, out_cb, ag_fire=None,
                         nhead=3):
                mtiles_all = {}

                def emit_gathers(sbi, q):
                    off, gn = goff[(sbi, q)]
                    if gn == 0:
                        return
                    idxt = pool.tile([P, gn_max * 8], I16, tag="idx",
                                     bufs=14)
                    nc.scalar.dma_start(
                        out=idxt[:, :gn * 8],
                        in_=t_idx[:, off * 8:(off + gn) * 8])
                    # deep-buffer the gather stream: with only ~1 group
                    # of tiles the gathers inherit every downstream
                    # bubble (isolated gather rate is ~3x the in-kernel
                    # rate at bufs=5)
                    mt = pool.tile([P, gn_max, F1], BF16, tag="mq",
                                   bufs=14)
                    if cfg.NOGATHER:
                        nc.gpsimd.memset(mt[:, :gn, :], 0.5)
                    else:
                        # HW wedges above 1024 idxs/call (65 ring
                        # entries); cap chunks per call
                        GMAX = cfg.GMAX
                        for g0 in range(0, gn, GMAX):
                            gw = min(GMAX, gn - g0)
                            nc.gpsimd.dma_gather(
                                out_ap=mt[:, g0:g0 + gw, :],
                                in_ap=tabq[q][:],
                                idxs_ap=idxt[:, g0 * 8:(g0 + gw) * 8],
                                num_idxs=gw * P, num_idxs_reg=gw * P,
                                elem_size=F1,
                                queue_num=qctr[0] % cfg.QROT)
                            qctr[0] += 1
                    mtiles_all[(sbi, q)] = mt

                # head: interleave the first nhead groups' gathers
                # quarter-major with the AG dispatches, so the Pool stream
                # never queues a gather behind a later quarter's AG
                # input-wait
                nh = min(nhead, len(sbs)) if ag_fire is not None else 0
                for q in range(NQ):
                    if ag_fire is not None:
                        ag_fire(q)
                    for sbi in range(nh):
                        emit_gathers(sbi, q)

                for sbi, sb in enumerate(sbs):
                    if sbi >= nh:
                        for q in range(NQ):
                            emit_gathers(sbi, q)
                    mtiles = {q: mtiles_all.pop((sbi, q))
                              for q in range(NQ) if (sbi, q) in mtiles_all}
                    dsb = pool.tile([P, nchb_sb_max], BF16, tag="dstl")
                    d0 = int(doff[sb[0]])
                    nsb = nchb_sb[sbi]
                    nc.scalar.dma_start(
                        out=dsb[:, :nsb], in_=t_dstl[:, d0:d0 + nsb])
                    # local table rows for the group's blocks (self-loop term)
                    tbsb = pool.tile([P, SBSZ, F1], BF16, tag="tblk", bufs=2)
                    nc.sync.dma_start(
                        out=tbsb[:, :len(sb), :],
                        in_=tab_shard[sb[0] * P:(sb[-1] + 1) * P, :].rearrange(
                            "(c p) f -> p c f", p=P))
                    if cfg.NOMM:
                        continue
                    for b in sb:
                        nb_ch = int(nchb[b])
                        lo = int(doff[b]) - d0
                        tblk = tbsb[:, b - sb[0], :]
                        oh = pool.tile([P, P, nchb_max], BF16, tag="oh")
                        if cfg.NOONEHOT:
                            nc.vector.memset(oh[:, :, :nb_ch], 0.001)
                        else:
                            # all-packed APs -> DVE 2x mode
                            nc.vector.tensor_tensor(
                                out=oh[:, :, :nb_ch],
                                in0=dsb[:, None, lo:lo + nb_ch].to_broadcast(
                                    [P, P, nb_ch]),
                                in1=iota_rep[:, :, :nb_ch],
                                op=mybir.AluOpType.is_equal)
                        agg = psum.tile([P, P], F32, tag="agg",
                                        bufs=cfg.PSB)
                        j = 0
                        for q in range(NQ):
                            for i in range(int(nch[b][q])):
                                m = mtiles[q][:, boff[(b, q)] + i, :]
                                o = oh[:, :, j]
                                if layer == 1:
                                    nc.tensor.matmul(
                                        out=agg[:], lhsT=o, rhs=m,
                                        start=(j == 0), stop=False)
                                else:
                                    nc.tensor.matmul(
                                        out=agg[:], lhsT=m, rhs=o,
                                        start=(j == 0), stop=False)
                                j += 1
                        # self-loop: agg += I^T @ tblk (node-major) or
                        # tblk^T @ I (feature-major)
                        if layer == 1:
                            nc.tensor.matmul(
                                out=agg[:], lhsT=ident[:], rhs=tblk[:],
                                start=(nb_ch == 0), stop=True)
                        else:
                            nc.tensor.matmul(
                                out=agg[:], lhsT=tblk[:], rhs=ident[:],
                                start=(nb_ch == 0), stop=True)
                        out_cb(b, agg)

            # ---- L1: node-major agg; epilogue -> tab2_shard (+ chunked AG2)
            l1_stage = {}
            qmark = np.searchsorted(cfg.QLO[1:], (np.arange(NB) + 1) * P)
            qend = set(int(q) // P - 1 for q in cfg.QLO[1:])
            # issue each AG2 chunk one group after its quarter completes so
            # the t2 flush has landed (epilogues are on ACT, so the wait is
            # short)
            ag2_at = {}
            for qq, qe in enumerate(sorted(qend)):
                ag2_at.setdefault(min(qe + SBSZ, NB - 1), []).append(qq)

            def ag1_fire(q):
                if cfg.SKIPAG:
                    return
                qlo, qhi = int(cfg.QLO[q]), int(cfg.QLO[q + 1])
                nc.gpsimd.collective_compute(
                    "AllGather", mybir.AluOpType.bypass,
                    ins=[tab1_shard[qlo:qhi, :].opt()],
                    outs=[tab1_fq[q][:].opt()],
                    replica_groups=[list(range(NCORES))])
            l1_plo = [0]  # first block not yet flushed to DRAM

            def l1_out(b, agg):
                if b % SBSZ == 0:
                    l1_stage[b // SBSZ] = stage.tile([P, SBSZ, F1], BF16,
                                                     tag="t2", name="t2")
                t2 = l1_stage[b // SBSZ]
                if cfg.ZB1:
                    # t2 = relu(dinv*agg + 0)*dinv = relu(dinv2*agg); one ACT
                    nc.scalar.activation(
                        t2[:, b % SBSZ, :], agg[:],
                        mybir.ActivationFunctionType.Relu,
                        scale=dinv2c[:, b:b + 1])
                else:
                    v = pool.tile([P, F1], F32, tag="v")
                    nc.vector.scalar_tensor_tensor(
                        out=v[:], in0=agg[:], scalar=dinvc[:, b:b + 1],
                        in1=b1b[:], op0=mybir.AluOpType.mult,
                        op1=mybir.AluOpType.add)
                    r = pool.tile([P, F1], F32, tag="r")
                    nc.scalar.activation(r[:], v[:],
                                         mybir.ActivationFunctionType.Relu)
                    nc.vector.tensor_scalar_mul(t2[:, b % SBSZ, :], r[:],
                                                dinvc[:, b:b + 1])
                if b % SBSZ == SBSZ - 1 or b == NB - 1 or b in qend:
                    plo = l1_plo[0]
                    nfb = b - plo + 1
                    nc.sync.dma_start(
                        out=tab2_shard[plo * P:(plo + nfb) * P, :].rearrange(
                            "(c p) f -> p c f", p=P),
                        in_=t2[:, plo % SBSZ:plo % SBSZ + nfb, :])
                    l1_plo[0] = b + 1
                # fire pending AG2 chunks (delayed past their quarter end)
                if cfg.PHASES >= 2 and not cfg.SKIPAG:
                    for q in ag2_at.get(b, []):
                        qlo, qhi = int(cfg.QLO[q]), int(cfg.QLO[q + 1])
                        nc.gpsimd.collective_compute(
                            "AllGather", mybir.AluOpType.bypass,
                            ins=[tab2_shard[qlo:qhi, :].opt()],
                            outs=[tab2_fq[q][:].opt()],
                            replica_groups=[list(range(NCORES))])

            if cfg.PHASES >= 1:
                agg_pass(1, tab1_fq, tab1_shard, l1_out, ag_fire=ag1_fire)

            # ---- L2: feature-major agg; epilogue -> y
            l2_stage = {}

            def l2_out(b, agg):
                w = pool.tile([P, P], BF16, tag="w")
                if cfg.ZB2:
                    # defer the dinv_d scale to after the W2 matmul (it is a
                    # per-dst scalar); PSUM->SBUF cast rides the ACT copy
                    nc.scalar.copy(w[:], agg[:])
                else:
                    nc.vector.tensor_tensor(
                        out=w[:], in0=agg[:], in1=dinvb[:, b * P:(b + 1) * P],
                        op=mybir.AluOpType.mult)
                o2f = psum.tile([P, F1], F32, tag="aux")
                o2 = o2f[:, :F2]
                nc.tensor.matmul(out=o2, lhsT=w[:], rhs=W2_bf[:],
                                 start=True, stop=True)
                if b % SBSZ == 0:
                    l2_stage[b // SBSZ] = stage.tile([P, SBSZ, F2], F32,
                                                     tag="ys", name="ys")
                ys = l2_stage[b // SBSZ]
                if cfg.ZB2:
                    nc.scalar.mul(ys[:, b % SBSZ, :], o2[:],
                                  dinvc[:, b:b + 1])
                else:
                    nc.vector.tensor_add(ys[:, b % SBSZ, :], o2[:], b2b[:])
                if b % SBSZ == SBSZ - 1 or b == NB - 1:
                    blo = (b // SBSZ) * SBSZ
                    nfb = b - blo + 1
                    nc.scalar.dma_start(
                        out=t_y[blo * P:(blo + nfb) * P, :].rearrange(
                            "(c p) f -> p c f", p=P),
                        in_=ys[:, :nfb, :])

            if cfg.PHASES >= 2:
                agg_pass(2, tab2_fq, tab2_shard, l2_out)
            if cfg.NOMM:
                zt = pool.tile([P, F2], F32, tag="dbg0")
                nc.vector.memset(zt[:], 0.0)
                for bb in range(0, SH, P):
                    nc.scalar.dma_start(out=t_y[bb:bb + P, :], in_=zt[:])
            if cfg.PHASES < 2:
                # debug exit: y <- copy of tab1_full0 head
                dbt = pool.tile([P, F2], BF16, tag="dbgb")
                nc.sync.dma_start(out=dbt[:], in_=tab1_fq[0][0:P, 0:F2])
                dbg = pool.tile([P, F2], F32, tag="dbg")
                nc.vector.tensor_copy(dbg[:], dbt[:])
                for bb in range(0, SH, P):
                    nc.scalar.dma_start(out=t_y[bb:bb + P, :], in_=dbg[:])

    nc.compile()
    return nc


def _prep(x, edge_index, W1, b1, W2, b2, cfg=DEFAULT_CFG):
    N, SH, NB, NPAD = cfg.N, cfg.SH, cfg.NB, cfg.NPAD
    src0 = np.asarray(edge_index[0], dtype=np.int64)
    dst0 = np.asarray(edge_index[1], dtype=np.int64)
    # degree includes the self loop (matches reference)
    deg0 = (np.bincount(dst0, minlength=N) + 1).astype(np.float32)
    perm = _balanced_perm(deg0, cfg)
    perm = _refine_perm(perm, src0, dst0, cfg)
    src = perm[src0]
    dst = perm[dst0]
    degp = np.ones(NPAD, dtype=np.float32)  # pad nodes: deg 1 (no edges)
    degp[perm] = deg0
    plan = _plan(src, dst, cfg)

    xb = np.asarray(x, dtype=np.float32)
    W1b = np.asarray(W1, dtype=ml_dtypes.bfloat16)
    W2b = np.asarray(W2, dtype=ml_dtypes.bfloat16)
    b1b = np.ascontiguousarray(np.tile(np.asarray(b1, np.float32)[None, :], (P, 1)))
    b2b = np.ascontiguousarray(np.tile(np.asarray(b2, np.float32)[None, :], (P, 1)))

    # x rows in new order: xp[newid] = x[oldid]
    invp = np.empty(NPAD, dtype=np.int64)
    invp.fill(0)
    invp[perm] = np.arange(N)
    has = np.zeros(NPAD, dtype=bool)
    has[perm] = True

    in_maps = []
    for c in range(NCORES):
        rows = invp[c * SH:(c + 1) * SH]
        mask = has[c * SH:(c + 1) * SH]
        xs = xb[rows] * mask[:, None]
        xT = np.ascontiguousarray(xs.T.astype(ml_dtypes.bfloat16))
        degsh = degp[c * SH:(c + 1) * SH]
        degc = np.ascontiguousarray(degsh.reshape(NB, P).T)
        idx1, dstl = _core_arrays(plan, c, cfg)
        in_maps.append({
            "xT": xT, "W1": W1b, "W2": W2b, "b1b": b1b, "b2b": b2b,
            "degc": degc, "degr": degsh, "idx": idx1, "dstl": dstl,
        })
    return plan, in_maps, perm


def _get_program(plan, cfg=DEFAULT_CFG):
    key = (hashlib.sha256(plan["nch"].tobytes()).hexdigest() +
           f"{cfg.N}_{cfg.PHASES}_{cfg.NOGATHER}_{cfg.NOONEHOT}_{cfg.SKIPAG}_{cfg.GMAX}_{cfg.QROT}_{cfg.SBSZ}_{cfg.PSB}_{cfg.NOMM}_{cfg.LOCALTAB}_{cfg.ZB1}_{cfg.ZB2}_{cfg.WG}_v4")
    if key not in _cache:
        _cache[key] = _build_program(plan, cfg)
    return _cache[key]


def _make_runner(nc, cfg):
    """Persistent jitted SPMD executor (mirrors bass2jax.run_bass_via_pjrt's
    multi-core path) so repeated calls reuse the compiled NEFF."""
    import jax
    from jax.sharding import Mesh, PartitionSpec
    from jax.experimental.shard_map import shard_map
    from concourse import bass2jax as b2j

    b2j.install_neuronx_cc_hook()
    assert nc.dbg_addr is None
    partition_name = (nc.partition_id_tensor.name
                      if nc.partition_id_tensor else None)

    in_names, out_names, out_avals = [], [], []
    for alloc in nc.m.functions[0].allocations:
        if not isinstance(alloc, mybir.MemoryLocationSet):
            continue
        name = alloc.memorylocations[0].name
        if alloc.kind == "ExternalInput":
            if name != partition_name:
                in_names.append(name)
        elif alloc.kind == "ExternalOutput":
            out_names.append(name)
            out_avals.append(jax.core.ShapedArray(
                tuple(alloc.tensor_shape), mybir.dt.np(alloc.dtype)))
    n_params = len(in_names)
    n_outs = len(out_names)
    all_names = in_names + out_names
    if partition_name is not None:
        all_names = all_names + [partition_name]
    donate = tuple(range(n_params, n_params + n_outs))

    def _body(*args):
        operands = list(args)
        if partition_name is not None:
            operands.append(b2j.partition_id_tensor())
        outs = b2j._bass_exec_p.bind(
            *operands,
            out_avals=tuple(out_avals),
            in_names=tuple(all_names),
            out_names=tuple(out_names),
            lowering_input_output_aliases=(),
            sim_require_finite=True,
            sim_require_nnan=True,
            nc=nc,
        )
        return tuple(outs)

    devices = jax.devices()[:NCORES]
    mesh = Mesh(np.asarray(devices), ("core",))
    sharded = jax.jit(
        shard_map(_body, mesh=mesh,
                  in_specs=(PartitionSpec("core"),) * (n_params + n_outs),
                  out_specs=(PartitionSpec("core"),) * n_outs,
                  check_rep=False),
        donate_argnums=donate, keep_unused=True)
    return {
        "fn": sharded, "in_names": in_names, "out_names": out_names,
        "out_avals": out_avals, "mesh": mesh,
    }


def _runner_args(runner, in_maps):
    concat_in = [
        np.concatenate([np.asarray(in_maps[c][k]) for c in range(NCORES)], 0)
        for k in runner["in_names"]
    ]
    zeros = [
        np.zeros((NCORES * a.shape[0],) + tuple(a.shape[1:]), a.dtype)
        for a in runner["out_avals"]
    ]
    return concat_in, zeros


def _get_runner(plan, cfg=DEFAULT_CFG):
    key = ("runner_" + hashlib.sha256(plan["nch"].tobytes()).hexdigest() +
           f"{cfg.N}_{cfg.PHASES}_{cfg.NOGATHER}_{cfg.NOONEHOT}_{cfg.SKIPAG}_{cfg.GMAX}_{cfg.QROT}_{cfg.SBSZ}_{cfg.PSB}_{cfg.NOMM}_{cfg.LOCALTAB}_{cfg.ZB1}_{cfg.ZB2}_{cfg.WG}_v4")
    if key not in _cache:
        _cache[key] = _make_runner(_get_program(plan, cfg), cfg)
    return _cache[key]


def _cfg_for(b1, b2):
    zb1 = bool(np.all(np.asarray(b1) == 0))
    zb2 = bool(np.all(np.asarray(b2) == 0))
    if zb1 and zb2:
        return DEFAULT_CFG
    return CFG(N=100000, IN_DIM=512, ZB1=zb1, ZB2=zb2)


def kernel(x, edge_index, W1, b1, W2, b2):
    cfg = _cfg_for(b1, b2)
    plan, in_maps, perm = _prep(x, edge_index, W1, b1, W2, b2, cfg)
    runner = _get_runner(plan, cfg)
    concat_in, zeros = _runner_args(runner, in_maps)
    outs = runner["fn"](*concat_in, *zeros)
    y = np.asarray(outs[runner["out_names"].index("y")]).reshape(cfg.NPAD, F2)
    return np.ascontiguousarray(y[perm])


def _floor_overhead(iters=10):
    """Per-call dispatch floor of this PJRT/axon path (trivial program)."""
    import time
    import jax
    from jax.sharding import NamedSharding, PartitionSpec
    import concourse.tile as tile_mod

    if "floor_nc" not in _cache:
        nc = bacc.Bacc("TRN2", target_bir_lowering=False, debug=False,
                       num_devices=NCORES)
        x = nc.declare_dram_parameter("x", [P, P], F32, isOutput=False)
        y = nc.declare_dram_parameter("y", [P, P], F32, isOutput=True)
        with tile_mod.TileContext(nc) as tc:
            with tc.tile_pool(name="sbuf", bufs=2) as pool:
                t = pool.tile([P, P], F32)
                nc.sync.dma_start(out=t[:], in_=x[:])
                t2 = pool.tile([P, P], F32)
                nc.vector.tensor_scalar_mul(t2[:], t[:], 3.0)
                nc.sync.dma_start(out=y[:], in_=t2[:])
        nc.compile()
        _cache["floor_nc"] = _make_runner(nc, None)
    runner = _cache["floor_nc"]
    in_maps = [{"x": np.zeros((P, P), np.float32)} for _ in range(NCORES)]
    concat_in, zeros = _runner_args(runner, in_maps)
    sh = NamedSharding(runner["mesh"], PartitionSpec("core"))
    dev_in = [jax.device_put(a, sh) for a in concat_in]
    zsets = [[jax.device_put(z, sh) for z in zeros] for _ in range(iters + 1)]
    jax.block_until_ready([dev_in, zsets])
    jax.block_until_ready(runner["fn"](*dev_in, *zsets[0]))
    ts = []
    for i in range(iters):
        t0 = time.perf_counter()
        jax.block_until_ready(runner["fn"](*dev_in, *zsets[i + 1]))
        ts.append(time.perf_counter() - t0)
    return min(ts)


def benchmark(inputs, iters=5):
    """Device-time estimate via pipelined-dispatch slope.

    The axon/PJRT channel latency is large and bimodal, so single-call
    wall-minus-floor is unusable. Instead dispatch n back-to-back
    executions (device runs them contiguously) and fit the slope:
    est = (min T(n_hi) - min T(n_lo)) / (n_hi - n_lo), which cancels the
    per-batch channel overhead.
    """
    import time
    import jax
    from jax.sharding import NamedSharding, PartitionSpec

    cfg = _cfg_for(inputs["b1"], inputs["b2"])
    plan, in_maps, _perm = _prep(**inputs, cfg=cfg)
    runner = _get_runner(plan, cfg)
    concat_in, zeros = _runner_args(runner, in_maps)
    sh = NamedSharding(runner["mesh"], PartitionSpec("core"))
    dev_in = [jax.device_put(a, sh) for a in concat_in]

    def run_n(n):
        zsets = [[jax.device_put(z, sh) for z in zeros] for _ in range(n)]
        jax.block_until_ready(zsets)
        t0 = time.perf_counter()
        outs = [runner["fn"](*dev_in, *zsets[i]) for i in range(n)]
        jax.block_until_ready(outs)
        return time.perf_counter() - t0

    run_n(1)  # warmup (compile)
    n_lo, n_hi = 1, 17
    t_lo, t_hi = [], []
    reps = max(4, (iters + 1) // 2)
    for _ in range(reps):
        t_lo.append(run_n(n_lo))
        t_hi.append(run_n(n_hi))
    est = (min(t_hi) - min(t_lo)) / (n_hi - n_lo)
    floor = min(t_lo)
    raw = min(t_lo)
    return {"raw_ns": int(raw * 1e9), "floor_ns": int(floor * 1e9),
            "est_ns": int(max(est, raw * 0.001) * 1e9)}



# revision 55
# speedup vs baseline: 2.8739x; 2.6967x over previous
"""Two-layer GCN (PyG GCNConv x2 + ReLU) on 8 Trainium2 NeuronCores.

Sharding: nodes are balance-relabeled and partitioned across the 8 cores
(12544 each incl. pad; N padded 100000 -> 100352 = 8*98*128). Each core:
  S0: h1 = x_shard @ W1 (bf16 matmul), p1 = dinv*h1 -> bf16 table shard
      (PSUM->SBUF scale rides an ACT copy); written quarter-by-quarter,
      each quarter AllGathered as soon as ready (4 chunked AllGathers ->
      chunk-major replicated table, overlapping the collective with the S0
      tail + L1 head).
  L1: for each dst-block of 128 nodes, gather p1[src] rows for its incoming
      non-self edges (dma_gather, int16 indices per <=29696-row segment
      aligned with the AG chunks, 1024 idxs/call rotated across 4 SWDGE
      queues - the single-queue descriptor path is the kernel's bottleneck),
      build 0/1 selector tiles on DVE with an all-packed-AP is_equal (2x DVE
      mode), segment-sum via TensorE matmuls in PSUM; the self-loop term is
      added with one identity matmul from the local table block (no gather).
      Epilogue relu(dinv^2*agg) (zero-bias fast path, one ACT op) -> bf16
      table2 shard, AllGathered in 4 quarter chunks that fire mid-L1 and
      overlap with the L1 tail / L2 head.
  L2: same gather/selector pass over table2 (feature-major accumulation),
      then out = dinv*(agg2 @ W2) -> fp32 output shard (dinv applied after
      the W2 matmul as a per-partition ACT scale).
Host reassembles the 8 output shards and inverts the relabeling.

Edges are grouped per (core, dst-block, src-chunk) with chunk counts made
uniform across cores so a single SPMD program serves all 8 cores; padding
slots gather row 0 and carry a -1 dst that the selector maps to zero.
The first nhead=3 groups' gathers are emitted quarter-major, interleaved
with the AG dispatches, so no gather ever queues behind a later quarter's
AG input-wait on the in-order Pool engine. Nonzero b1/b2 fall back to a
general (DVE) epilogue path chosen at program-build time.
"""

import hashlib
import sys

for _p in ("/opt/trn_rl_repo",):
    if _p not in sys.path:
        sys.path.insert(0, _p)

import numpy as np
import ml_dtypes

import concourse.bass as bass  # noqa: F401  (engine types via nc)
import concourse.bacc as bacc
import concourse.mybir as mybir
import concourse.tile as tile

BF16 = mybir.dt.bfloat16
F32 = mybir.dt.float32
I16 = mybir.dt.int16
I32 = mybir.dt.int32

P = 128
NCORES = 8
F1 = 128
F2 = 64
NQ = 4


class CFG:
    def __init__(self, N, IN_DIM, SBSZ=6, MSPAN=1280, PHASES=2,
                 NOGATHER=False, NOONEHOT=False, SKIPAG=False,
                 GMAX=8, QROT=4, PSB=2, NOMM=False, LOCALTAB=True,
                 ZB1=True, ZB2=True, WG=3):
        # WG: sb-groups per wave. Blocks of a wave keep persistent PSUM
        # accumulators across the 4 src-quarters so gather tiles free per
        # quarter (not per 4-quarter group), and AllGathers interleave with
        # wave-0 quarters.
        self.WG = WG
        # ZB1/ZB2: bias vectors known to be all-zero (host-checked) -> the
        # whole epilogue collapses to one ACT op per block (scale folded in).
        self.ZB1 = ZB1
        self.ZB2 = ZB2
        self.PHASES = PHASES
        self.NOGATHER = NOGATHER
        self.NOONEHOT = NOONEHOT
        self.SKIPAG = SKIPAG
        self.GMAX = GMAX
        self.QROT = QROT
        self.PSB = PSB
        self.NOMM = NOMM
        self.LOCALTAB = LOCALTAB
        self.N = N
        self.NPAD = NCORES * 98 * P            # 100352
        self.SH = self.NPAD // NCORES          # 12544
        self.NB = self.SH // P                 # 98
        # per-shard quarter row counts (block-aligned), sum = SH. The last
        # quarter is oversized so its edge cells target ~604 of a 640
        # (5-chunk) cap while the others target ~479 of 512 (4 chunks) --
        # this gives the balance packer ~1.3 sigma of slack per cell.
        # Small quarters FIRST: AG chunk 0 of each table covers fewer rows,
        # completes sooner, and unblocks the L1/L2 gather streams earlier.
        self.QROWS = [2944, 2944, 2944, 3712]
        self.QLO = np.concatenate([[0], np.cumsum(self.QROWS)]).astype(np.int64)
        # per-chunk full-table segment sizes (8 * qrows), all < 32768
        self.SEGSZ = [NCORES * r for r in self.QROWS]
        self.IN_DIM = IN_DIM
        self.SBSZ = SBSZ
        self.MSPAN = MSPAN


DEFAULT_CFG = CFG(N=100000, IN_DIM=512)

_cache = {}


def _balanced_perm(deg, cfg):
    """Relabel nodes so per-(core,block) in-degree sums are balanced.

    Returns perm: old node id -> new node id in [0, NPAD).
    New id layout: core c owns [c*SH, (c+1)*SH); block b of core c is
    rows [c*SH + b*P, c*SH + (b+1)*P).
    """
    NPAD, SH, NB = cfg.NPAD, cfg.SH, cfg.NB
    nbins = NCORES * NB
    order = np.argsort(-deg, kind="stable")  # heavy nodes first
    # snake-deal node ranks into bins: round r covers bins in alternating order
    nodes_per_bin = P
    perm = np.empty(cfg.N, dtype=np.int64)
    fwd = np.arange(nbins)
    bwd = fwd[::-1]
    pos_in_bin = np.zeros(nbins, dtype=np.int64)
    idx = 0
    r = 0
    npts = len(order)
    while idx < npts:
        bins = fwd if (r % 2 == 0) else bwd
        take = min(nbins, npts - idx)
        sel = order[idx:idx + take]
        b = bins[:take]
        # new id: bin b -> core = b // NB, block = b % NB
        core = b // NB
        blk = b % NB
        perm[sel] = core * SH + blk * P + pos_in_bin[b]
        pos_in_bin[b] += 1
        idx += take
        r += 1
    assert pos_in_bin.max() <= nodes_per_bin
    return perm


def _refine_perm(perm, src0, dst0, cfg):
    """Re-bin nodes within each (core, quarter) to equalize the per
    (core, dst-block, src-quarter) edge-cell counts, minimizing the padded
    chunk count sum(ceil(max_core(cell)/128)).

    Quarter membership (and hence every edge's src-quarter) is invariant
    under these moves, so cell profiles can be computed once.
    """
    SH, NB, NPAD = cfg.SH, cfg.NB, cfg.NPAD
    qlo = cfg.QLO
    qblk = [int(q) // P for q in qlo]          # block index at quarter starts
    src = perm[src0]
    dst = perm[dst0]
    # per-node in-profile over src quarters (invariant)
    sq = np.searchsorted(qlo[1:-1], src % SH, side="right")
    pin = np.zeros((NPAD, NQ), dtype=np.int64)
    np.add.at(pin, (dst, sq), 1)

    new_perm_pos = np.arange(NPAD, dtype=np.int64)  # new position per new id

    def pack(order_ids, prof, nblk, caps):
        """Greedy: place nodes (given order) into nblk blocks, cap P nodes
        each, minimizing hinge over caps [nblk, NQ]."""
        fill = np.zeros((nblk, NQ), dtype=np.int64)
        cnt = np.zeros(nblk, dtype=np.int64)
        assign = np.empty(len(order_ids), dtype=np.int64)
        for i, v in enumerate(order_ids):
            p = prof[i]
            over = np.maximum(fill + p - caps, 0) - np.maximum(fill - caps, 0)
            score = over.sum(axis=1).astype(np.float64)
            # tie-break: prefer emptier blocks (balance node counts)
            score += cnt * 1e-6
            score[cnt >= P] = np.inf
            b = int(np.argmin(score))
            assign[i] = b
            fill[b] += p
            cnt[b] += 1
        return assign, fill

    # two rounds: first against the (5,4,4,4)-chunk grid matched to the
    # skewed quarter sizes, then against the chunk grid actually paid for
    # (max over cores), letting overflow consolidate into paid cells.
    caps_all = np.tile(np.array([[4, 4, 4, 5]], dtype=np.int64) * P, (NB, 1))
    for rnd in range(2):
        fills = np.zeros((NCORES, NB, NQ), dtype=np.int64)
        for c in range(NCORES):
            for Q in range(NQ):
                blo, bhi = qblk[Q], qblk[Q + 1]
                ids = np.arange(c * SH + qlo[Q], c * SH + qlo[Q + 1])
                prof = pin[ids]
                o = np.argsort(-prof.sum(axis=1), kind="stable")
                ids, prof = ids[o], prof[o]
                assign, fill = pack(ids, prof, bhi - blo, caps_all[blo:bhi])
                fills[c, blo:bhi] = fill
                # positions: stable order within block
                order2 = np.argsort(assign, kind="stable")
                srt = assign[order2]
                startb = np.searchsorted(srt, np.arange(bhi - blo))
                posn = c * SH + (blo + srt) * P + (np.arange(len(ids)) -
                                                   startb[srt])
                new_perm_pos[ids[order2]] = posn
        caps_all = np.ceil(fills.max(axis=0) / P).astype(np.int64) * P
    # compose: old id -> phase1 new id -> refined position
    return new_perm_pos[perm]


def _plan(src, dst, cfg):
    """Group (non-self) edges by (core, dst-block, src-chunk).

    src/dst are NEW (relabeled) node ids. Returns the uniform chunk plan.
    """
    SH, NB = cfg.SH, cfg.NB
    c = dst // SH
    dloc = dst - c * SH
    b = dloc // P
    dl = dloc - b * P
    # src chunk + index within chunk segment
    sc = src % SH
    q = np.searchsorted(cfg.QLO[1:-1], sc, side="right")
    iseg = (src // SH) * np.asarray(cfg.QROWS)[q] + (sc - cfg.QLO[q])
    key = ((c * NB + b) * NQ + q).astype(np.int64)
    counts = np.bincount(key, minlength=NCORES * NB * NQ).reshape(NCORES, NB, NQ)
    order = np.argsort(key, kind="stable")
    starts = np.zeros(NCORES * NB * NQ + 1, dtype=np.int64)
    np.cumsum(counts.reshape(-1), out=starts[1:])
    nch = np.ceil(counts.max(axis=0) / P).astype(np.int64)  # [NB, NQ] uniform
    sbs = [list(range(i, min(i + cfg.SBSZ, NB))) for i in range(0, NB, cfg.SBSZ)]
    return {
        "order": order, "starts": starts, "counts": counts,
        "nch": nch, "sbs": sbs, "iseg": iseg, "dl": dl,
    }


def _core_arrays(plan, core, cfg):
    """Build idx (gather stream, (sb,q,b) order) + dstl ((wave,q,b)-major)."""
    nch, sbs = plan["nch"], plan["sbs"]
    order, starts = plan["order"], plan["starts"]
    iseg, dl = plan["iseg"], plan["dl"]
    NB = cfg.NB

    cell_iv = {}
    cell_dv = {}
    for b in range(NB):
        for q in range(NQ):
            n_ch = nch[b][q]
            if n_ch == 0:
                continue
            k = (core * NB + b) * NQ + q
            sl = order[starts[k]:starts[k + 1]]
            # ascending source rows within the cell: consecutive gather
            # descriptors walk the segment in address order (HBM page
            # locality for the latency-bound random reads)
            sl = sl[np.argsort(iseg[sl], kind="stable")]
            pad = n_ch * P - len(sl)
            cell_iv[(b, q)] = np.concatenate([iseg[sl], np.zeros(pad, np.int64)])
            cell_dv[(b, q)] = np.concatenate([dl[sl], np.full(pad, -1, np.int64)])

    idx_cols = []   # per (sb,q): [16, gn*8] int16 segments
    for sb in sbs:
        for q in range(NQ):
            vals = [cell_iv[(b, q)] for b in sb if (b, q) in cell_iv]
            if vals:
                v = np.concatenate(vals)
                idx_cols.append(v.reshape(-1, 16).T.astype(np.int16))
    idx1 = np.tile(np.concatenate(idx_cols, axis=1), (8, 1)) if idx_cols else \
        np.zeros((128, 0), np.int16)

    # dstl columns block-major (per block: its 4 quarters' cells in order)
    dstl_parts = []
    for b in range(NB):
        for q in range(NQ):
            if (b, q) in cell_dv:
                dstl_parts.append(cell_dv[(b, q)].reshape(-1, P).T)
    dstl = np.concatenate(dstl_parts, axis=1).astype(np.float32)
    return np.ascontiguousarray(idx1), \
        np.ascontiguousarray(dstl.astype(ml_dtypes.bfloat16))


def _build_program(plan, cfg):
    SH, NB = cfg.SH, cfg.NB
    IN_DIM, SBSZ, MSPAN = cfg.IN_DIM, cfg.SBSZ, cfg.MSPAN
    KC = IN_DIM // P
    nch, sbs = plan["nch"], plan["sbs"]
    nchb = nch.sum(axis=1)                      # chunks per block
    totch = int(nchb.sum())
    nchb_max = int(nchb.max())
    # gather-stream offsets per (sbi, q) and per-block offsets within groups
    goff = {}
    boff = {}
    off = 0
    for sbi, sb in enumerate(sbs):
        for q in range(NQ):
            gn = int(sum(nch[b][q] for b in sb))
            goff[(sbi, q)] = (off, gn)
            o = 0
            for b in sb:
                boff[(b, q)] = o
                o += int(nch[b][q])
            off += gn
    gn_max = max(gn for (_, gn) in goff.values())
    doff = np.zeros(NB + 1, dtype=np.int64)
    np.cumsum(nchb, out=doff[1:])
    nchb_sb = [int(sum(nchb[b] for b in sb)) for sb in sbs]
    nchb_sb_max = max(nchb_sb)

    nc = bacc.Bacc("TRN2", target_bir_lowering=False, debug=False,
                   num_devices=NCORES, num_swdge_queues=min(4, max(1, cfg.QROT)))
    t_xT = nc.declare_dram_parameter("xT", [IN_DIM, SH], BF16, isOutput=False)
    t_W1 = nc.declare_dram_parameter("W1", [IN_DIM, F1], BF16, isOutput=False)
    t_W2 = nc.declare_dram_parameter("W2", [F1, F2], BF16, isOutput=False)
    t_b1b = nc.declare_dram_parameter("b1b", [P, F1], F32, isOutput=False)
    t_b2b = nc.declare_dram_parameter("b2b", [P, F2], F32, isOutput=False)
    t_degc = nc.declare_dram_parameter("degc", [P, NB], F32, isOutput=False)
    t_degr = (None if cfg.ZB2 else
              nc.declare_dram_parameter("degr", [NB * P], F32, isOutput=False))
    t_idx = nc.declare_dram_parameter("idx", [P, totch * 8], I16, isOutput=False)
    t_dstl = nc.declare_dram_parameter("dstl", [P, totch], BF16, isOutput=False)
    t_y = nc.declare_dram_parameter("y", [SH, F2], F32, isOutput=True)

    # Local (non-Shared) collective outputs: dma_gather reads from the
    # Shared scratchpad run ~28% slower per descriptor (~+1ms over the
    # kernel), and the collectives have plenty of slack to take the
    # non-Shared path instead.
    _aspace = "Local" if cfg.LOCALTAB else "Shared"
    tab1_fq = [nc.dram_tensor(f"tab1_full{q}", [cfg.SEGSZ[q], F1], BF16,
                              addr_space=_aspace) for q in range(NQ)]
    tab2_fq = [nc.dram_tensor(f"tab2_full{q}", [cfg.SEGSZ[q], F1], BF16,
                              addr_space=_aspace) for q in range(NQ)]

    with tile.TileContext(nc) as tc:
        with (
            tc.tile_pool(name="dram", bufs=1, space="DRAM") as dram,
            tc.tile_pool(name="consts", bufs=1) as consts,
            tc.tile_pool(name="sb", bufs=3) as pool,
            tc.tile_pool(name="stage", bufs=2) as stage,
            tc.tile_pool(name="psum", bufs=2, space="PSUM") as psum,
        ):
            tab1_shard = dram.tile([SH, F1], BF16)
            tab2_shard = dram.tile([SH, F1], BF16)

            # ---- constants
            iota_i = consts.tile([P, P], I32)
            nc.gpsimd.iota(iota_i[:], pattern=[[1, P]], base=0, channel_multiplier=0)
            iota_p = consts.tile([P, P], I32)
            nc.gpsimd.iota(iota_p[:], pattern=[[0, P]], base=0, channel_multiplier=1)
            iota_bf = consts.tile([P, P], BF16)
            nc.vector.tensor_copy(iota_bf[:], iota_i[:])
            ident = consts.tile([P, P], BF16)
            nc.vector.tensor_tensor(out=ident[:], in0=iota_i[:], in1=iota_p[:],
                                    op=mybir.AluOpType.is_equal)
            iota_rep = consts.tile([P, P, nchb_max], BF16)
            nc.vector.tensor_copy(
                iota_rep[:],
                iota_bf[:, :, None].to_broadcast([P, P, nchb_max]))

            W1_sb = consts.tile([P, KC, F1], BF16)
            nc.sync.dma_start(out=W1_sb[:],
                              in_=t_W1[:].rearrange("(c p) f -> p c f", p=P))
            W2_bf = consts.tile([P, F2], BF16)
            nc.sync.dma_start(out=W2_bf[:], in_=t_W2[:])
            b1b = consts.tile([P, F1], F32)
            nc.sync.dma_start(out=b1b[:], in_=t_b1b[:])
            b2b = consts.tile([P, F2], F32)
            nc.sync.dma_start(out=b2b[:], in_=t_b2b[:])

            degc = consts.tile([P, NB], F32)
            nc.sync.dma_start(out=degc[:], in_=t_degc[:])
            sq = consts.tile([P, NB], F32)
            nc.scalar.sqrt(sq[:], degc[:])
            dinvc = consts.tile([P, NB], F32)
            nc.vector.reciprocal(dinvc[:], sq[:])
            # dinv^2 per (node, block): relu(dinv*agg)*dinv == relu(dinv2*agg)
            dinv2c = consts.tile([P, NB], F32)
            nc.vector.reciprocal(dinv2c[:], degc[:])

            dinvb = None
            if not cfg.ZB2:
                dinvb = consts.tile([P, NB * P], BF16)
                DSPAN = 1568
                for dspan in range(0, NB * P, DSPAN):
                    dw = min(DSPAN, NB * P - dspan)
                    degb_t = pool.tile([P, DSPAN], F32, tag="degb")
                    nc.sync.dma_start(
                        out=degb_t[:, :dw],
                        in_=t_degr[None, dspan:dspan + dw].to_broadcast([P, dw]))
                    sqb_t = pool.tile([P, DSPAN], F32, tag="sqb")
                    nc.scalar.sqrt(sqb_t[:, :dw], degb_t[:, :dw])
                    rec_t = pool.tile([P, DSPAN], F32, tag="recb")
                    nc.vector.reciprocal(rec_t[:, :dw], sqb_t[:, :dw])
                    nc.vector.tensor_copy(dinvb[:, dspan:dspan + dw], rec_t[:, :dw])

            # ---- S0: h1 = x @ W1 (node-major), p1 = dinv*h1 -> tab1_shard
            # quarter-by-quarter; AllGather each quarter as soon as written.
            for q in range(NQ):
                qlo, qhi = int(cfg.QLO[q]), int(cfg.QLO[q + 1])
                for s0 in range(qlo, qhi, MSPAN):
                    mw = min(MSPAN, qhi - s0)
                    nsub = mw // P
                    xt = pool.tile([P, KC, MSPAN], BF16, tag="xT", bufs=2)
                    nc.sync.dma_start(
                        out=xt[:, :, :mw],
                        in_=t_xT[:, s0:s0 + mw].rearrange("(c p) m -> p c m", p=P))
                    p1s = stage.tile([P, MSPAN // P, F1], BF16, tag="p1s")
                    for sub in range(nsub):
                        moff = sub * P
                        hps = psum.tile([P, F1], F32, tag="aux")
                        for kc in range(KC):
                            nc.tensor.matmul(
                                out=hps[:],
                                lhsT=xt[:, kc, moff:moff + P],
                                rhs=W1_sb[:, kc, :],
                                start=(kc == 0), stop=(kc == KC - 1))
                        B = (s0 + moff) // P
                        nc.scalar.mul(p1s[:, sub, :], hps[:], dinvc[:, B:B + 1])
                    nc.sync.dma_start(
                        out=tab1_shard[s0:s0 + mw, :].rearrange(
                            "(c p) f -> p c f", p=P),
                        in_=p1s[:, :nsub, :])
                # AG1 dispatch deferred into the L1 pass (quarter-major head
                # interleave): see ag1_fire below.

            # ---- aggregation pass (shared for L1/L2)
            qctr = [0]  # global gather-queue rotation (balanced across groups)

            def agg_pass(layer, tabq, tab_shard, out_cb, ag_fire=None,
                         nhead=3):
                mtiles_all = {}

                def emit_gathers(sbi, q):
                    off, gn = goff[(sbi, q)]
                    if gn == 0:
                        return
                    idxt = pool.tile([P, gn_max * 8], I16, tag="idx",
                                     bufs=14)
                    nc.scalar.dma_start(
                        out=idxt[:, :gn * 8],
                        in_=t_idx[:, off * 8:(off + gn) * 8])
                    # deep-buffer the gather stream: with only ~1 group
                    # of tiles the gathers inherit every downstream
                    # bubble (isolated gather rate is ~3x the in-kernel
                    # rate at bufs=5)
                    mt = pool.tile([P, gn_max, F1], BF16, tag="mq",
                                   bufs=14)
                    if cfg.NOGATHER:
                        nc.gpsimd.memset(mt[:, :gn, :], 0.5)
                    else:
                        # HW wedges above 1024 idxs/call (65 ring
                        # entries); cap chunks per call
                        GMAX = cfg.GMAX
                        for g0 in range(0, gn, GMAX):
                            gw = min(GMAX, gn - g0)
                            nc.gpsimd.dma_gather(
                                out_ap=mt[:, g0:g0 + gw, :],
                                in_ap=tabq[q][:],
                                idxs_ap=idxt[:, g0 * 8:(g0 + gw) * 8],
                                num_idxs=gw * P, num_idxs_reg=gw * P,
                                elem_size=F1,
                                queue_num=qctr[0] % cfg.QROT)
                            qctr[0] += 1
                    mtiles_all[(sbi, q)] = mt

                # head: interleave the first nhead groups' gathers
                # quarter-major with the AG dispatches, so the Pool stream
                # never queues a gather behind a later quarter's AG
                # input-wait
                nh = min(nhead, len(sbs)) if ag_fire is not None else 0
                for q in range(NQ):
                    if ag_fire is not None:
                        ag_fire(q)
                    for sbi in range(nh):
                        emit_gathers(sbi, q)

                for sbi, sb in enumerate(sbs):
                    if sbi >= nh:
                        for q in range(NQ):
                            emit_gathers(sbi, q)
                    mtiles = {q: mtiles_all.pop((sbi, q))
                              for q in range(NQ) if (sbi, q) in mtiles_all}
                    dsb = pool.tile([P, nchb_sb_max], BF16, tag="dstl")
                    d0 = int(doff[sb[0]])
                    nsb = nchb_sb[sbi]
                    nc.scalar.dma_start(
                        out=dsb[:, :nsb], in_=t_dstl[:, d0:d0 + nsb])
                    # local table rows for the group's blocks (self-loop term)
                    tbsb = pool.tile([P, SBSZ, F1], BF16, tag="tblk", bufs=2)
                    nc.sync.dma_start(
                        out=tbsb[:, :len(sb), :],
                        in_=tab_shard[sb[0] * P:(sb[-1] + 1) * P, :].rearrange(
                            "(c p) f -> p c f", p=P))
                    if cfg.NOMM:
                        continue
                    for b in sb:
                        nb_ch = int(nchb[b])
                        lo = int(doff[b]) - d0
                        tblk = tbsb[:, b - sb[0], :]
                        oh = pool.tile([P, P, nchb_max], BF16, tag="oh")
                        if cfg.NOONEHOT:
                            nc.vector.memset(oh[:, :, :nb_ch], 0.001)
                        else:
                            # all-packed APs -> DVE 2x mode
                            nc.vector.tensor_tensor(
                                out=oh[:, :, :nb_ch],
                                in0=dsb[:, None, lo:lo + nb_ch].to_broadcast(
                                    [P, P, nb_ch]),
                                in1=iota_rep[:, :, :nb_ch],
                                op=mybir.AluOpType.is_equal)
                        agg = psum.tile([P, P], F32, tag="agg",
                                        bufs=cfg.PSB)
                        j = 0
                        for q in range(NQ):
                            for i in range(int(nch[b][q])):
                                m = mtiles[q][:, boff[(b, q)] + i, :]
                                o = oh[:, :, j]
                                if layer == 1:
                                    nc.tensor.matmul(
                                        out=agg[:], lhsT=o, rhs=m,
                                        start=(j == 0), stop=False)
                                else:
                                    nc.tensor.matmul(
                                        out=agg[:], lhsT=m, rhs=o,
                                        start=(j == 0), stop=False)
                                j += 1
                        # self-loop: agg += I^T @ tblk (node-major) or
                        # tblk^T @ I (feature-major)
                        if layer == 1:
                            nc.tensor.matmul(
                                out=agg[:], lhsT=ident[:], rhs=tblk[:],
                                start=(nb_ch == 0), stop=True)
                        else:
                            nc.tensor.matmul(
                                out=agg[:], lhsT=tblk[:], rhs=ident[:],
                                start=(nb_ch == 0), stop=True)
                        out_cb(b, agg)

            # ---- L1: node-major agg; epilogue -> tab2_shard (+ chunked AG2)
            l1_stage = {}
            qmark = np.searchsorted(cfg.QLO[1:], (np.arange(NB) + 1) * P)
            qend = set(int(q) // P - 1 for q in cfg.QLO[1:])
            # issue each AG2 chunk one group after its quarter completes so
            # the t2 flush has landed (epilogues are on ACT, so the wait is
            # short)
            ag2_at = {}
            for qq, qe in enumerate(sorted(qend)):
                ag2_at.setdefault(min(qe + SBSZ, NB - 1), []).append(qq)

            def ag1_fire(q):
                if cfg.SKIPAG:
                    return
                qlo, qhi = int(cfg.QLO[q]), int(cfg.QLO[q + 1])
                nc.gpsimd.collective_compute(
                    "AllGather", mybir.AluOpType.bypass,
                    ins=[tab1_shard[qlo:qhi, :].opt()],
                    outs=[tab1_fq[q][:].opt()],
                    replica_groups=[list(range(NCORES))])
            l1_plo = [0]  # first block not yet flushed to DRAM

            def l1_out(b, agg):
                if b % SBSZ == 0:
                    l1_stage[b // SBSZ] = stage.tile([P, SBSZ, F1], BF16,
                                                     tag="t2", name="t2")
                t2 = l1_stage[b // SBSZ]
                if cfg.ZB1:
                    # t2 = relu(dinv*agg + 0)*dinv = relu(dinv2*agg); one ACT
                    nc.scalar.activation(
                        t2[:, b % SBSZ, :], agg[:],
                        mybir.ActivationFunctionType.Relu,
                        scale=dinv2c[:, b:b + 1])
                else:
                    v = pool.tile([P, F1], F32, tag="v")
                    nc.vector.scalar_tensor_tensor(
                        out=v[:], in0=agg[:], scalar=dinvc[:, b:b + 1],
                        in1=b1b[:], op0=mybir.AluOpType.mult,
                        op1=mybir.AluOpType.add)
                    r = pool.tile([P, F1], F32, tag="r")
                    nc.scalar.activation(r[:], v[:],
                                         mybir.ActivationFunctionType.Relu)
                    nc.vector.tensor_scalar_mul(t2[:, b % SBSZ, :], r[:],
                                                dinvc[:, b:b + 1])
                if b % SBSZ == SBSZ - 1 or b == NB - 1 or b in qend:
                    plo = l1_plo[0]
                    nfb = b - plo + 1
                    nc.sync.dma_start(
                        out=tab2_shard[plo * P:(plo + nfb) * P, :].rearrange(
                            "(c p) f -> p c f", p=P),
                        in_=t2[:, plo % SBSZ:plo % SBSZ + nfb, :])
                    l1_plo[0] = b + 1
                # fire pending AG2 chunks (delayed past their quarter end)
                if cfg.PHASES >= 2 and not cfg.SKIPAG:
                    for q in ag2_at.get(b, []):
                        qlo, qhi = int(cfg.QLO[q]), int(cfg.QLO[q + 1])
                        nc.gpsimd.collective_compute(
                            "AllGather", mybir.AluOpType.bypass,
                            ins=[tab2_shard[qlo:qhi, :].opt()],
                            outs=[tab2_fq[q][:].opt()],
                            replica_groups=[list(range(NCORES))])

            if cfg.PHASES >= 1:
                agg_pass(1, tab1_fq, tab1_shard, l1_out, ag_fire=ag1_fire)

            # ---- L2: feature-major agg; epilogue -> y
            l2_stage = {}

            def l2_out(b, agg):
                w = pool.tile([P, P], BF16, tag="w")
                if cfg.ZB2:
                    # defer the dinv_d scale to after the W2 matmul (it is a
                    # per-dst scalar); PSUM->SBUF cast rides the ACT copy
                    nc.scalar.copy(w[:], agg[:])
                else:
                    nc.vector.tensor_tensor(
                        out=w[:], in0=agg[:], in1=dinvb[:, b * P:(b + 1) * P],
                        op=mybir.AluOpType.mult)
                o2f = psum.tile([P, F1], F32, tag="aux")
                o2 = o2f[:, :F2]
                nc.tensor.matmul(out=o2, lhsT=w[:], rhs=W2_bf[:],
                                 start=True, stop=True)
                if b % SBSZ == 0:
                    l2_stage[b // SBSZ] = stage.tile([P, SBSZ, F2], F32,
                                                     tag="ys", name="ys")
                ys = l2_stage[b // SBSZ]
                if cfg.ZB2:
                    nc.scalar.mul(ys[:, b % SBSZ, :], o2[:],
                                  dinvc[:, b:b + 1])
                else:
                    nc.vector.tensor_add(ys[:, b % SBSZ, :], o2[:], b2b[:])
                if b % SBSZ == SBSZ - 1 or b == NB - 1:
                    blo = (b // SBSZ) * SBSZ
                    nfb = b - blo + 1
                    nc.scalar.dma_start(
                        out=t_y[blo * P:(blo + nfb) * P, :].rearrange(
                            "(c p) f -> p c f", p=P),
                        in_=ys[:, :nfb, :])

            if cfg.PHASES >= 2:
                agg_pass(2, tab2_fq, tab2_shard, l2_out)
            if cfg.NOMM:
                zt = pool.tile([P, F2], F32, tag="dbg0")
                nc.vector.memset(zt[:], 0.0)
                for bb in range(0, SH, P):
                    nc.scalar.dma_start(out=t_y[bb:bb + P, :], in_=zt[:])
            if cfg.PHASES < 2:
                # debug exit: y <- copy of tab1_full0 head
                dbt = pool.tile([P, F2], BF16, tag="dbgb")
                nc.sync.dma_start(out=dbt[:], in_=tab1_fq[0][0:P, 0:F2])
                dbg = pool.tile([P, F2], F32, tag="dbg")
                nc.vector.tensor_copy(dbg[:], dbt[:])
                for bb in range(0, SH, P):
                    nc.scalar.dma_start(out=t_y[bb:bb + P, :], in_=dbg[:])

    nc.compile()
    return nc


def _prep(x, edge_index, W1, b1, W2, b2, cfg=DEFAULT_CFG):
    N, SH, NB, NPAD = cfg.N, cfg.SH, cfg.NB, cfg.NPAD
    src0 = np.asarray(edge_index[0], dtype=np.int64)
    dst0 = np.asarray(edge_index[1], dtype=np.int64)
    # degree includes the self loop (matches reference)
    deg0 = (np.bincount(dst0, minlength=N) + 1).astype(np.float32)
    perm = _balanced_perm(deg0, cfg)
    perm = _refine_perm(perm, src0, dst0, cfg)
    src = perm[src0]
    dst = perm[dst0]
    degp = np.ones(NPAD, dtype=np.float32)  # pad nodes: deg 1 (no edges)
    degp[perm] = deg0
    plan = _plan(src, dst, cfg)

    xb = np.asarray(x, dtype=np.float32)
    W1b = np.asarray(W1, dtype=ml_dtypes.bfloat16)
    W2b = np.asarray(W2, dtype=ml_dtypes.bfloat16)
    b1b = np.ascontiguousarray(np.tile(np.asarray(b1, np.float32)[None, :], (P, 1)))
    b2b = np.ascontiguousarray(np.tile(np.asarray(b2, np.float32)[None, :], (P, 1)))

    # x rows in new order: xp[newid] = x[oldid]
    invp = np.empty(NPAD, dtype=np.int64)
    invp.fill(0)
    invp[perm] = np.arange(N)
    has = np.zeros(NPAD, dtype=bool)
    has[perm] = True

    in_maps = []
    for c in range(NCORES):
        rows = invp[c * SH:(c + 1) * SH]
        mask = has[c * SH:(c + 1) * SH]
        xs = xb[rows] * mask[:, None]
        xT = np.ascontiguousarray(xs.T.astype(ml_dtypes.bfloat16))
        degsh = degp[c * SH:(c + 1) * SH]
        degc = np.ascontiguousarray(degsh.reshape(NB, P).T)
        idx1, dstl = _core_arrays(plan, c, cfg)
        in_maps.append({
            "xT": xT, "W1": W1b, "W2": W2b, "b1b": b1b, "b2b": b2b,
            "degc": degc, "degr": degsh, "idx": idx1, "dstl": dstl,
        })
    return plan, in_maps, perm


def _get_program(plan, cfg=DEFAULT_CFG):
    key = (hashlib.sha256(plan["nch"].tobytes()).hexdigest() +
           f"{cfg.N}_{cfg.PHASES}_{cfg.NOGATHER}_{cfg.NOONEHOT}_{cfg.SKIPAG}_{cfg.GMAX}_{cfg.QROT}_{cfg.SBSZ}_{cfg.PSB}_{cfg.NOMM}_{cfg.LOCALTAB}_{cfg.ZB1}_{cfg.ZB2}_{cfg.WG}_v4")
    if key not in _cache:
        _cache[key] = _build_program(plan, cfg)
    return _cache[key]


def _make_runner(nc, cfg):
    """Persistent jitted SPMD executor (mirrors bass2jax.run_bass_via_pjrt's
    multi-core path) so repeated calls reuse the compiled NEFF."""
    import jax
    from jax.sharding import Mesh, PartitionSpec
    from jax.experimental.shard_map import shard_map
    from concourse import bass2jax as b2j

    b2j.install_neuronx_cc_hook()
    assert nc.dbg_addr is None
    partition_name = (nc.partition_id_tensor.name
                      if nc.partition_id_tensor else None)

    in_names, out_names, out_avals = [], [], []
    for alloc in nc.m.functions[0].allocations:
        if not isinstance(alloc, mybir.MemoryLocationSet):
            continue
        name = alloc.memorylocations[0].name
        if alloc.kind == "ExternalInput":
            if name != partition_name:
                in_names.append(name)
        elif alloc.kind == "ExternalOutput":
            out_names.append(name)
            out_avals.append(jax.core.ShapedArray(
                tuple(alloc.tensor_shape), mybir.dt.np(alloc.dtype)))
    n_params = len(in_names)
    n_outs = len(out_names)
    all_names = in_names + out_names
    if partition_name is not None:
        all_names = all_names + [partition_name]
    donate = tuple(range(n_params, n_params + n_outs))

    def _body(*args):
        operands = list(args)
        if partition_name is not None:
            operands.append(b2j.partition_id_tensor())
        outs = b2j._bass_exec_p.bind(
            *operands,
            out_avals=tuple(out_avals),
            in_names=tuple(all_names),
            out_names=tuple(out_names),
            lowering_input_output_aliases=(),
            sim_require_finite=True,
            sim_require_nnan=True,
            nc=nc,
        )
        return tuple(outs)

    devices = jax.devices()[:NCORES]
    mesh = Mesh(np.asarray(devices), ("core",))
    sharded = jax.jit(
        shard_map(_body, mesh=mesh,
                  in_specs=(PartitionSpec("core"),) * (n_params + n_outs),
                  out_specs=(PartitionSpec("core"),) * n_outs,
                  check_rep=False),
        donate_argnums=donate, keep_unused=True)
    return {
        "fn": sharded, "in_names": in_names, "out_names": out_names,
        "out_avals": out_avals, "mesh": mesh,
    }


def _runner_args(runner, in_maps):
    concat_in = [
        np.concatenate([np.asarray(in_maps[c][k]) for c in range(NCORES)], 0)
        for k in runner["in_names"]
    ]
    zeros = [
        np.zeros((NCORES * a.shape[0],) + tuple(a.shape[1:]), a.dtype)
        for a in runner["out_avals"]
    ]
    return concat_in, zeros


def _get_runner(plan, cfg=DEFAULT_CFG):
    key = ("runner_" + hashlib.sha256(plan["nch"].tobytes()).hexdigest() +
           f"{cfg.N}_{cfg.PHASES}_{cfg.NOGATHER}_{cfg.NOONEHOT}_{cfg.SKIPAG}_{cfg.GMAX}_{cfg.QROT}_{cfg.SBSZ}_{cfg.PSB}_{cfg.NOMM}_{cfg.LOCALTAB}_{cfg.ZB1}_{cfg.ZB2}_{cfg.WG}_v4")
    if key not in _cache:
        _cache[key] = _make_runner(_get_program(plan, cfg), cfg)
    return _cache[key]


def _cfg_for(b1, b2):
    zb1 = bool(np.all(np.asarray(b1) == 0))
    zb2 = bool(np.all(np.asarray(b2) == 0))
    if zb1 and zb2:
        return DEFAULT_CFG
    return CFG(N=100000, IN_DIM=512, ZB1=zb1, ZB2=zb2)


def kernel(x, edge_index, W1, b1, W2, b2):
    cfg = _cfg_for(b1, b2)
    plan, in_maps, perm = _prep(x, edge_index, W1, b1, W2, b2, cfg)
    runner = _get_runner(plan, cfg)
    concat_in, zeros = _runner_args(runner, in_maps)
    outs = runner["fn"](*concat_in, *zeros)
    y = np.asarray(outs[runner["out_names"].index("y")]).reshape(cfg.NPAD, F2)
    return np.ascontiguousarray(y[perm])


def _floor_overhead(iters=10):
    """Per-call dispatch floor of this PJRT/axon path (trivial program)."""
    import time
    import jax
    from jax.sharding import NamedSharding, PartitionSpec
    import concourse.tile as tile_mod

    if "floor_nc" not in _cache:
        nc = bacc.Bacc("TRN2", target_bir_lowering=False, debug=False,
                       num_devices=NCORES)
        x = nc.declare_dram_parameter("x", [P, P], F32, isOutput=False)
        y = nc.declare_dram_parameter("y", [P, P], F32, isOutput=True)
        with tile_mod.TileContext(nc) as tc:
            with tc.tile_pool(name="sbuf", bufs=2) as pool:
                t = pool.tile([P, P], F32)
                nc.sync.dma_start(out=t[:], in_=x[:])
                t2 = pool.tile([P, P], F32)
                nc.vector.tensor_scalar_mul(t2[:], t[:], 3.0)
                nc.sync.dma_start(out=y[:], in_=t2[:])
        nc.compile()
        _cache["floor_nc"] = _make_runner(nc, None)
    runner = _cache["floor_nc"]
    in_maps = [{"x": np.zeros((P, P), np.float32)} for _ in range(NCORES)]
    concat_in, zeros = _runner_args(runner, in_maps)
    sh = NamedSharding(runner["mesh"], PartitionSpec("core"))
    dev_in = [jax.device_put(a, sh) for a in concat_in]
    zsets = [[jax.device_put(z, sh) for z in zeros] for _ in range(iters + 1)]
    jax.block_until_ready([dev_in, zsets])
    jax.block_until_ready(runner["fn"](*dev_in, *zsets[0]))
    ts = []
    for i in range(iters):
        t0 = time.perf_counter()
        jax.block_until_ready(runner["fn"](*dev_in, *zsets[i + 1]))
        ts.append(time.perf_counter() - t0)
    return min(ts)


def benchmark(inputs, iters=5):
    """Device-time measurement.

    Primary: neuron-profile (NTFF) capture of one traced execution — the
    true on-device exec time, immune to axon/PJRT channel behavior.
    Fallback: pipelined-dispatch slope (the channel latency is large and
    bimodal, so single-call wall-minus-floor is unusable; dispatch n
    back-to-back executions and fit the slope, cancelling the per-batch
    channel overhead).
    """
    import time
    import jax
    from jax.sharding import NamedSharding, PartitionSpec

    cfg = _cfg_for(inputs["b1"], inputs["b2"])
    plan, in_maps, _perm = _prep(**inputs, cfg=cfg)

    try:
        import tempfile
        from concourse import bass_utils
        nc = _get_program(plan, cfg)
        res = bass_utils.run_bass_kernel_spmd(
            nc, in_maps, core_ids=list(range(NCORES)),
            tmpdir=tempfile.mkdtemp(), trace=True, trace_cores=[0])
        if res.exec_time_ns:
            t = int(res.exec_time_ns)
            return {"raw_ns": t, "floor_ns": 0, "est_ns": t}
    except Exception:
        pass

    runner = _get_runner(plan, cfg)
    concat_in, zeros = _runner_args(runner, in_maps)
    sh = NamedSharding(runner["mesh"], PartitionSpec("core"))
    dev_in = [jax.device_put(a, sh) for a in concat_in]

    def run_n(n):
        zsets = [[jax.device_put(z, sh) for z in zeros] for _ in range(n)]
        jax.block_until_ready(zsets)
        t0 = time.perf_counter()
        outs = [runner["fn"](*dev_in, *zsets[i]) for i in range(n)]
        jax.block_until_ready(outs)
        return time.perf_counter() - t0

    run_n(1)  # warmup (compile)
    n_lo, n_hi = 1, 17
    t_lo, t_hi = [], []
    reps = max(4, (iters + 1) // 2)
    for _ in range(reps):
        t_lo.append(run_n(n_lo))
        t_hi.append(run_n(n_hi))
    est = (min(t_hi) - min(t_lo)) / (n_hi - n_lo)
    floor = min(t_lo)
    raw = min(t_lo)
    return {"raw_ns": int(raw * 1e9), "floor_ns": int(floor * 1e9),
            "est_ns": int(max(est, raw * 0.001) * 1e9)}

